# revision 50
# baseline (speedup 1.0000x reference)
"""Trainium2 Bass kernel for AttentionMLPReduction (fp8 DoubleRow version).

Reference computation (per sample, B=256, L=32, H=4096, E=2048, NH=8, hd=256):
  h    = relu(x @ w_red.T + b_red)                  (B,L,E)
  qkv  = h @ w_in.T + b_in ; q,k,v = split(qkv)
  attn = softmax(q @ k.T / sqrt(hd))  per head      (B,NH,L,L)
  ctx  = attn @ v                                   (B,NH,L,hd) -> (B,L,E)
  attn_output = ctx @ w_out.T + b_out               (B,L,E)
  w_mean = attn.mean(heads); w_norm = w_mean / rowsum  (== w_mean)
  pooled = mean_q(w_norm @ attn_output)             (B,E)
  out = sigmoid(mlp(pooled))                        (B,1)

Algebraic simplifications:
  * w_norm == w_mean exactly (rows already sum to 1).
  * pooled[b] = u[b] @ attn_output[b] with u[b,l] = mean_q w_mean[b,q,l].
  * z[b] := u[b] @ ctx[b]; pooled = z @ w_out.T + b_out_eff with
    b_out_eff = b_out + w_out @ b_in_v (since sum_l u[b,l] == 1).
  * w_out is folded into the MLP head entirely:
      o1 = relu(pooled @ w1.T + b1) = relu(z @ (w1 @ w_out).T + b1_eff),
      b1_eff = b1 + w1 @ b_out_eff, so the (B,E)x(E,E) GEMM3 disappears.

Precision: the three big GEMMs (x@w_red, h@w_qk, h@w_v) run in fp8 e4m3 with
MatmulPerfMode.DoubleRow (2 k-rows per PE pass). Host pre-scales operands to
the e4m3 sweet range with power-of-2 factors (exact), and the dequant scales
fold into the psum->sbuf activation step:
  x*32, w_red*4096 -> h8 = 16*h via relu scale 1/8192
  h8 (=16*h), w_in*8192 -> q,k,v via scale 1/131072
Softmax block-diag masking is folded into the scores psum as a rank-4 matmul
driving off-diagonal 32x32 blocks to exp(-100) == 0. Schedule: GEMM1 (DMA
issue order tuned so x/wred sub-tiles arrive just-in-time), v GEMM, then per
head-pair qk GEMMs with the previous pair's attention interleaved 1:1 into
the fp8 streams; z is fused into the last pair's tails; the final sigmoid
avoids an ACT_TABLE_LOAD by using exp + reciprocal.

Sharding: pure data parallel over batch; 32 samples per core, weights
replicated (prepared/cast once on the host).

Per-core layouts (partition dim first):
  xT8   [H=4096, M=1024] fp8   (M = 32 samples x L=32), value = 32*x
  hT    [E=2048, M]      fp8   in SBUF, value = 16*h
  qT,kT per head-pair: [128, 4*M] fp8 (col = dtile*M + m), value = 32*q
    -> scores run as one fp8-DR matmul over both d-tiles per group
  v     [M, E]           bf16   natural orientation
  ctx   [M, E]           bf16
  zT    [E, 32]          bf16 -> o1T [256,32] -> ... -> outT [1,32]
"""

import os
import numpy as np
import ml_dtypes

import concourse.bass as bass
import concourse.mybir as mybir
import concourse.tile as tile
from concourse import bacc
from concourse.bass_utils import run_bass_kernel_spmd
from concourse.masks import make_identity

BF16 = mybir.dt.bfloat16
F8 = mybir.dt.float8e4
F32 = mybir.dt.float32
AF = mybir.ActivationFunctionType
DR = mybir.MatmulPerfMode.DoubleRow

B, L, H, E, NH = 256, 32, 4096, 2048, 8
HD = E // NH  # 256
NCORES = 8
BC = B // NCORES  # 32 samples per core
M = BC * L  # 1024 rows per core
P = 128
KX = H // P  # 32 k-tiles for GEMM1
KE = E // P  # 16 k-tiles for E-contraction GEMMs
MT = M // P  # 8 m-tiles
GS = P // L  # 4 samples per partition-tile

# fp8 scaling (all powers of two -> exact to fold/unfold)
XS = 32.0        # x pre-scale
WRS = 4096.0     # w_red pre-scale
HS = 16.0        # h post-scale (stored h8 = HS*h)
WIS = 8192.0     # w_in pre-scale
G1_SCALE = HS / (XS * WRS)      # psum -> h8
QKV_SCALE = 1.0 / (HS * WIS)    # psum -> q/k/v
SQK = 32.0   # q/k post-scale: qT/kT stored fp8 as 32*q (|q|<3.4 -> <109)

# module-level stash for the last run's HW exec time (ns), if traced
LAST_EXEC_TIME_NS = None


def _install_ntff_hook_shim():
    """antenv.axon_hooks is missing in this container; bass_utils imports it
    when trace=True under axon. Recreate it and register the ctypes-driven
    NRT profile hook from trn_boot if available."""
    import sys
    import types
    try:
        from antenv import axon_hooks  # noqa: F401
        return
    except ImportError:
        pass
    try:
        import antenv
    except ImportError:
        return
    m = types.ModuleType("antenv.axon_hooks")
    m._hook = None
    m.set_axon_ntff_profile_hook = lambda h: setattr(m, "_hook", h)
    m.get_axon_ntff_profile_hook = lambda: m._hook
    sys.modules["antenv.axon_hooks"] = m
    antenv.axon_hooks = m
    try:
        from trn_agent_boot.trn_boot import _ntff_profile_via_ctypes
        hook = _ntff_profile_via_ctypes("/opt/axon/libaxon_pjrt.so")
        if hook is not None:
            m._hook = hook
    except Exception:
        pass


def _build_kernel() -> bass.Bass:
    nc = bacc.Bacc(None, target_bir_lowering=False, debug=False)

    # ---- DRAM parameters (per-core shard views) ----
    # pre-striped on host: every row below is the exact SBUF line for one
    # partition, so DMAs are contiguous 8-16KB lines at full HBM speed.
    xS = nc.dram_tensor("xS", [P, KX * M], F8, kind="ExternalInput")
    wredS = nc.dram_tensor("wredS", [P, (KE // 2) * KX * 2 * P], F8,
                           kind="ExternalInput")
    winSq = nc.dram_tensor("winSq", [P, 4 * KE * 512], F8, kind="ExternalInput")
    winSk = nc.dram_tensor("winSk", [P, 4 * KE * 512], F8, kind="ExternalInput")
    winSv = nc.dram_tensor("winSv", [P, 4 * KE * 512], F8, kind="ExternalInput")
    w1S = nc.dram_tensor("w1S", [P, KE * 256], BF16, kind="ExternalInput")
    w2T = nc.dram_tensor("w2T", [256, P], BF16, kind="ExternalInput")
    w3T = nc.dram_tensor("w3T", [P, 64], BF16, kind="ExternalInput")
    w4T = nc.dram_tensor("w4T", [64, 1], BF16, kind="ExternalInput")
    bredT16 = nc.dram_tensor("bredT16", [P, KE], F32, kind="ExternalInput")
    binT = nc.dram_tensor("binT", [P, 2 * KE], F32, kind="ExternalInput")
    b1effT = nc.dram_tensor("b1effT", [P, 2], F32, kind="ExternalInput")
    b2T = nc.dram_tensor("b2T", [P, 1], F32, kind="ExternalInput")
    b3T = nc.dram_tensor("b3T", [64, 1], F32, kind="ExternalInput")
    b4 = nc.dram_tensor("b4", [1, 1], F32, kind="ExternalInput")
    mbl = nc.dram_tensor("mbl", [GS, P], BF16, kind="ExternalInput")
    mbr = nc.dram_tensor("mbr", [GS, P], BF16, kind="ExternalInput")
    out = nc.dram_tensor("out", [BC, 1], F32, kind="ExternalOutput")

    from contextlib import ExitStack

    with tile.TileContext(nc) as tc, ExitStack() as ctx:
        const = ctx.enter_context(tc.tile_pool(name="const", bufs=1))
        bredT_sb = const.tile([P, KE], F32)
        binT_sb = const.tile([P, 2 * KE], F32)
        b1T_sb = const.tile([P, 2], F32)
        b2T_sb = const.tile([P, 1], F32)
        b3T_sb = const.tile([64, 1], F32)
        b4_sb = const.tile([1, 1], F32)
        b4n_sb = const.tile([1, 1], F32)
        ind_sb = const.tile([P, GS], BF16)
        ident_sb = const.tile([P, P], BF16)
        mbl_sb = const.tile([GS, P], BF16)
        mbr_sb = const.tile([GS, P], BF16)

        def _const_dmas():
            # all consts ride the scalar ring AFTER the x chunks: the gpsimd
            # software ring only moves ~16KB/25us once the kernel is running
            # and its final trickle blocked kernel teardown by ~4us.
            nc.scalar.dma_start(binT_sb[:], binT[:])
            nc.scalar.dma_start(b1T_sb[:], b1effT[:])
            nc.scalar.dma_start(b2T_sb[:], b2T[:])
            nc.scalar.dma_start(b3T_sb[:], b3T[:])
            nc.scalar.dma_start(b4_sb[:], b4[:])
            nc.vector.tensor_scalar_mul(b4n_sb[:], b4_sb[:], -1.0)
            # indicator[p, j] = 1.0 if p // 32 == j else 0 (for z block-sums)
            nc.any.memset(ind_sb[:], 0.0)
            for j in range(GS):
                nc.any.memset(ind_sb[j * L:(j + 1) * L, j:j + 1], 1.0)
            make_identity(nc, ident_sb)
            # rank-4 factors of the block-diag mask bias: -1600*SQK^2 off-
            # diagonal, 0 on-diagonal (scores psum += mbl.T @ mbr).
            nc.scalar.dma_start(mbl_sb[:], mbl[:])
            nc.scalar.dma_start(mbr_sb[:], mbr[:])

        # persistent activations (live across phases)
        acts = ctx.enter_context(tc.tile_pool(name="acts", bufs=1))
        hT_sb = acts.tile([P, KE * M], F8)         # col = et*M + m, = 16*h
        zT_sb = acts.tile([P, KE * BC], BF16)      # col = ec*BC + s
        o1T_sb = acts.tile([P, 2 * BC], BF16)
        o2T_sb = acts.tile([P, BC], BF16)
        o3T_sb = acts.tile([64, BC], BF16)
        outT_sb = acts.tile([1, BC], F32)

        # x + weights streamed early. Scalar ring (Q10): xT, v stripes, w1
        # family, then ALL qk pair stripes (issued after GEMM1 so they never
        # queue ahead of wred stripes). Sync ring (Q1): GEMM1 eg stripes
        # only, bufs=3 with issue-ahead so eg arrivals always lead compute.
        # Gpsimd ring starts earliest (~5us): it carries the first-needed
        # x/wred sub-tiles so the first matmul can fire ~7us in.
        w1p = ctx.enter_context(tc.tile_pool(name="w1p", bufs=1))
        w1s = w1p.tile([P, KE * 256], BF16)
        w2_sb = w1p.tile([P, 2 * P], BF16)
        w3_sb = w1p.tile([P, 64], BF16)
        w4_sb = w1p.tile([64, 1], BF16)
        EG = 2  # e-tiles per wred stripe

        def _eg_dma(eg):
            stripe = wpool.tile([P, KX * EG * P], F8, name=f"wrs{eg}",
                                tag="wrs")
            sz = KX * EG * P
            nc.sync.dma_start(stripe[:], wredS[:, eg * sz:(eg + 1) * sz])
            return stripe

        wqk = ctx.enter_context(tc.tile_pool(name="winqk", bufs=2))
        vpool = ctx.enter_context(tc.tile_pool(name="vctx", bufs=1))
        v_sb = vpool.tile([P, MT * E], BF16)    # col = mt*E + f
        ctx_sb = vpool.tile([P, MT * E], BF16)  # col = mt*E + e
        wv_cm = tc.tile_pool(name="winv", bufs=1)
        wv = wv_cm.__enter__()
        v_stripes = [wv.tile([P, KE * 512], F8, name=f"vst{fc}", tag=f"vst{fc}")
                     for fc in range(4)]
        # wred + x pools are innermost so both can close (LIFO) right after
        # GEMM1, freeing 48KB/partition for the attention-phase pools.
        wpool_cm = tc.tile_pool(name="wred", bufs=2)
        wpool = wpool_cm.__enter__()
        xpool_cm = tc.tile_pool(name="xT", bufs=1)
        xpool = xpool_cm.__enter__()
        xq = [xpool.tile([P, (KX // 4) * M], F8, name=f"xq{i}", tag=f"xq{i}")
              for i in range(4)]

        # --- gpsimd software ring: small late-needed consts only (it moves
        # ~5KB in the first 6us, then only ~16KB/25us).
        _const_dmas()
        # --- scalar ring: bredT first (needed ~25us), then x in 0.5MB
        # halves so GEMM1's q-quarters never wait on a full 1MB chunk, then
        # the MLP tail weights (arrive ~25us; needed ~350us).
        QSZ = KE * 512
        XSZ = (KX // 4) * M
        for i in range(4):
            # kt-pair quarters for the first two chunks (finer arrival
            # granularity while DMA is oversubscribed), halves after.
            nsub = 4 if i < 2 else 2
            w = XSZ // nsub
            for hf in range(nsub):
                c0 = i * XSZ + hf * w
                nc.scalar.dma_start(xq[i][:, hf * w:(hf + 1) * w],
                                    xS[:, c0:c0 + w])
        nc.scalar.dma_start(
            w2_sb[:].rearrange("p (ke n) -> p ke n", ke=2),
            w2T[:].rearrange("(ke p) n -> p ke n", p=P))
        nc.scalar.dma_start(w3_sb[:], w3T[:])
        nc.scalar.dma_start(w4_sb[:], w4T[:])
        # --- sync ring: wred ONLY during GEMM1 (q0 quarter first so the
        # first matmuls wait on 256KB, not 1MB); v/w1 queue BEHIND the eg
        # stripes (emitted after the GEMM1 loop) so they can never starve
        # the wred stream mid-GEMM.
        wrs0 = wpool.tile([P, KX * EG * P], F8, name="wrs0", tag="wrs")
        kw = 8 * EG * P  # wred cols for kt0-7 (the q=0 quarter)
        nc.sync.dma_start(wrs0[:, 0:kw], wredS[:, 0:kw])
        nc.sync.dma_start(wrs0[:, kw:KX * EG * P], wredS[:, kw:KX * EG * P])
        # bredT (64B rows, packet-slow) hides behind wrs0 on the sync ring,
        # done ~14us, needed ~30us.
        nc.sync.dma_start(bredT_sb[:], bredT16[:])
        eg_stripes = {0: wrs0, 1: _eg_dma(1)}
        qk_stripes = {}  # (hp, 'qs'/'ks') -> tile; filled after GEMM1

        with ExitStack() as s1:
            xq3 = [t[:].rearrange("p (kt m) -> p kt m", kt=KX // 4) for t in xq]
            ps1 = s1.enter_context(tc.tile_pool(name="ps1", bufs=2, space="PSUM"))
            for eg in range(KE // EG):
                stripe = eg_stripes[eg]
                if eg + 2 < KE // EG:
                    eg_stripes[eg + 2] = _eg_dma(eg + 2)
                w3r = stripe[:].rearrange("p (kt e) -> p kt e", kt=KX)
                # 4 open psum groups (el, mc), accumulated in 4 quarter-k
                # passes so compute can start after the first x chunk lands.
                psums = {}
                for el in range(EG):
                    for mc in range(2):
                        psums[el, mc] = ps1.tile(
                            [P, 512], F32, name=f"g1ps{el}{mc}",
                            tag=f"g1ps{el}{mc}")
                for q in range(4):
                    for el in range(EG):
                        for mc in range(2):
                            for kp in range(4):
                                nc.tensor.matmul(
                                    psums[el, mc][:],
                                    w3r[:, q * 8 + 2 * kp:q * 8 + 2 * kp + 2,
                                        el * P:(el + 1) * P],
                                    xq3[q][:, 2 * kp:2 * kp + 2,
                                           mc * 512:(mc + 1) * 512],
                                    start=(q == 0 and kp == 0),
                                    stop=(q == 3 and kp == 3),
                                    perf_mode=DR)
                for el in range(EG):
                    et = eg * EG + el
                    for mc in range(2):
                        nc.scalar.activation(
                            hT_sb[:, et * M + mc * 512:et * M + (mc + 1) * 512],
                            psums[el, mc][:], AF.Relu,
                            bias=bredT_sb[:, et:et + 1], scale=G1_SCALE)
        xpool_cm.__exit__(None, None, None)  # xq dead after GEMM1
        wpool_cm.__exit__(None, None, None)  # wred stripes dead after GEMM1
        # v stripes + w1 ride the sync ring BEHIND the eg stripes (arrive
        # ~110us, needed at the v GEMM ~125us).
        for fc in range(4):
            nc.sync.dma_start(v_stripes[fc][:],
                              winSv[:, fc * QSZ:(fc + 1) * QSZ])
        nc.sync.dma_start(w1s[:], w1S[:])
        # qk pair-0/1 stripes on the now-idle scalar ring (arrive ~40us)
        for hp in range(2):
            for tag, src in (("qs", winSq), ("ks", winSk)):
                st = wqk.tile([P, KE * 512], F8, name=f"{tag}{hp}", tag=tag)
                nc.scalar.dma_start(st[:], src[:, hp * QSZ:(hp + 1) * QSZ])
                qk_stripes[(hp, tag)] = st
        h3 = hT_sb[:].rearrange("p (ke m) -> p ke m", ke=KE)

        # -------- qk GEMMs (all 4 pairs), then v GEMM + ALL attention ------
        # Attention needs v only as ctx-matmul rhs, so the entire attention
        # pipeline (scores/exp/softmax/u/ctx) interleaves into the v GEMM's
        # fp8 stream; z matmuls are deferred to one dense batch at the end
        # (back-to-back z mms pipeline at ~26ns each).
        with ExitStack() as s2:
            # ---- v GEMM first (attention ctx needs v); its psum pool
            # closes before the attention pools open.
            def _v_unit(psv, mt, fc):
                ps = psv.tile([P, 512], F32, name="vps", tag="vps")
                st3 = v_stripes[fc][:].rearrange("p (ke f) -> p ke f", ke=KE)
                for kp in range(KE // 2):
                    nc.tensor.matmul(
                        ps[:],
                        h3[:, 2 * kp:2 * kp + 2, mt * P:(mt + 1) * P],
                        st3[:, 2 * kp:2 * kp + 2, :],
                        start=(kp == 0), stop=(kp == KE // 2 - 1),
                        perf_mode=DR)
                dst = v_sb[:, mt * E + fc * 512:mt * E + (fc + 1) * 512]
                if fc % 2 == 0:
                    nc.scalar.activation(dst, ps[:], AF.Copy,
                                         scale=QKV_SCALE)
                else:
                    nc.vector.tensor_scalar_mul(dst, ps[:], QKV_SCALE)

            with ExitStack() as s2v:
                psv = s2v.enter_context(
                    tc.tile_pool(name="psv", bufs=4, space="PSUM"))
                for mt in range(MT):
                    for fc in range(4):
                        _v_unit(psv, mt, fc)
            wv_cm.__exit__(None, None, None)  # v stripes dead after v GEMM

            with ExitStack() as s2b:
                qk_out = s2b.enter_context(tc.tile_pool(name="qkT", bufs=4))
                psqk = s2b.enter_context(tc.tile_pool(name="psqk", bufs=2, space="PSUM"))
                psc = s2b.enter_context(tc.tile_pool(name="psc", bufs=2, space="PSUM"))
                psu = s2b.enter_context(tc.tile_pool(name="psu", bufs=1, space="PSUM"))
                attp = s2b.enter_context(tc.tile_pool(name="attp", bufs=4))
                atq = s2b.enter_context(tc.tile_pool(name="atq", bufs=3))
                # u accumulated across all heads in one psum bank; column
                # g's accumulation group spans h=0..7.
                psu_all = psu.tile([P, MT], F32, name="psu_all")

                def _qk_units(hp):
                    # 8 emission units (2 dst x 4 dl) for head-pair hp
                    if hp < 2:
                        q_stripe = qk_stripes[(hp, "qs")]
                        k_stripe = qk_stripes[(hp, "ks")]
                    else:
                        q_stripe = wqk.tile([P, KE * 512], F8, tag="qs")
                        k_stripe = wqk.tile([P, KE * 512], F8, tag="ks")
                        nc.scalar.dma_start(q_stripe[:],
                                            winSq[:, hp * QSZ:(hp + 1) * QSZ])
                        nc.scalar.dma_start(k_stripe[:],
                                            winSk[:, hp * QSZ:(hp + 1) * QSZ])
                    # qT2/kT2: col = dl*M + m, dl 0..3 (dtile = 4*hp + dl)
                    # stored fp8 (= SQK*q): scores run as ONE DoubleRow
                    # matmul per group, and all 4 pairs fit in SBUF.
                    qT_sb = qk_out.tile([P, 4 * M], F8, tag="qT")
                    kT_sb = qk_out.tile([P, 4 * M], F8, tag="kT")
                    units = []
                    for dst, stripe, bcol0 in ((qT_sb, q_stripe, 4 * hp),
                                               (kT_sb, k_stripe, KE + 4 * hp)):
                        s3 = stripe[:].rearrange("p (ke f) -> p ke f", ke=KE)
                        for dl in range(4):
                            def _u(dst=dst, s3=s3, bcol0=bcol0, dl=dl):
                                psums = [psqk.tile([P, 512], F32,
                                                   name=f"qkps{i}",
                                                   tag=f"qkps{i}")
                                         for i in range(2)]
                                for kp in range(KE // 2):
                                    for mc in range(2):
                                        nc.tensor.matmul(
                                            psums[mc][:],
                                            s3[:, 2 * kp:2 * kp + 2, dl * P:(dl + 1) * P],
                                            h3[:, 2 * kp:2 * kp + 2, mc * 512:(mc + 1) * 512],
                                            start=(kp == 0),
                                            stop=(kp == KE // 2 - 1),
                                            perf_mode=DR)
                                for mc in range(2):
                                    d_ap = dst[:, dl * M + mc * 512:dl * M + (mc + 1) * 512]
                                    if mc == 0:
                                        nc.scalar.activation(
                                            d_ap, psums[mc][:], AF.Identity,
                                            bias=binT_sb[:, bcol0 + dl:bcol0 + dl + 1],
                                            scale=QKV_SCALE * SQK)
                                    else:
                                        nc.vector.tensor_scalar(
                                            out=d_ap, in0=psums[mc][:],
                                            scalar1=QKV_SCALE * SQK,
                                            scalar2=binT_sb[:, bcol0 + dl:bcol0 + dl + 1],
                                            op0=mybir.AluOpType.mult,
                                            op1=mybir.AluOpType.add)
                            units.append(_u)
                    return units, qT_sb, kT_sb

                qkT = []

                # ---- v GEMM units: (mt, fc) granularity, fc-outer so only
                # one psum group is open at a time (2 banks via rotation).
                # ---- attention S/T closures per (head, group-batch) ----
                zref = {}

                def _mk_head(hp, hh):
                    h = 2 * hp + hh
                    d0 = 2 * hh
                    qT_sb, kT_sb = qkT[hp]
                    qv3 = qT_sb[:].rearrange("p (dl m) -> p dl m", dl=4)
                    kv3 = kT_sb[:].rearrange("p (dl m) -> p dl m", dl=4)

                    def _scores(gb):
                        g0 = gb * 4
                        ps_s4 = psqk.tile([P, 4 * P], F32,
                                          name=f"s4_{h}_{gb}", tag="qkps0")
                        for j in range(4):
                            g = g0 + j
                            sl = ps_s4[:, j * P:(j + 1) * P]
                            nc.tensor.matmul(
                                sl,
                                qv3[:, d0:d0 + 2, g * P:(g + 1) * P],
                                kv3[:, d0:d0 + 2, g * P:(g + 1) * P],
                                start=True, stop=False, perf_mode=DR)
                            nc.tensor.matmul(sl, mbl_sb[:], mbr_sb[:],
                                             start=False, stop=True)
                        # bd4 = block-diag masked exp(scores/sqrt(hd))
                        bd4 = attp.tile([P, 4 * P], BF16, tag="bd4")
                        nc.scalar.activation(
                            bd4[:], ps_s4[:], AF.Exp,
                            scale=1.0 / (np.sqrt(HD) * SQK * SQK))
                        S4 = attp.tile([P, 4], F32, tag="S4")
                        nc.vector.tensor_reduce(
                            S4[:],
                            bd4[:].rearrange("p (j q) -> p j q", j=4),
                            axis=mybir.AxisListType.X,
                            op=mybir.AluOpType.add)
                        rS4 = attp.tile([P, 4], F32, tag="rS4")
                        nc.vector.reciprocal(rS4[:], S4[:])
                        rS4b = attp.tile([P, 4], BF16, tag="rS4b")
                        nc.vector.tensor_copy(rS4b[:], rS4[:])
                        return bd4, rS4, rS4b

                    def _tail(st, gb):
                        bd4, rS4, rS4b = st
                        g0 = gb * 4
                        bdT4_ps = psqk.tile([P, 4 * P], BF16,
                                            name=f"bdT4_{h}_{gb}",
                                            tag="qkps1")
                        for j in range(4):
                            nc.tensor.transpose(
                                bdT4_ps[:, j * P:(j + 1) * P],
                                bd4[:, j * P:(j + 1) * P], ident_sb[:])
                        bdT4 = atq.tile([P, 4 * P], BF16, tag="bdT_sb")
                        nc.vector.tensor_copy(bdT4[:], bdT4_ps[:])
                        # u matmuls fill the bdT copy latency
                        for j in range(4):
                            nc.tensor.matmul(
                                psu_all[:, g0 + j:g0 + j + 1],
                                bd4[:, j * P:(j + 1) * P],
                                rS4b[:, j:j + 1],
                                start=(h == 0), stop=(h == NH - 1))
                        for jp in range(2):
                            ps_ctx2 = psc.tile([P, 2 * HD], F32, tag="ctx")
                            for jj in range(2):
                                j = jp * 2 + jj
                                g = g0 + j
                                nc.tensor.matmul(
                                    ps_ctx2[:, jj * HD:(jj + 1) * HD],
                                    bdT4[:, j * P:(j + 1) * P],
                                    v_sb[:, g * E + h * HD:g * E + (h + 1) * HD],
                                    start=True, stop=True)
                            for jj in range(2):
                                j = jp * 2 + jj
                                g = g0 + j
                                # normalized ctx copy+scale. During the sp
                                # phases scalar is loaded with qk acts, so
                                # it rides vector; in the exposed hp3 flush
                                # vector IS the bottleneck (its in-order
                                # queue stalls the next head's softmax), so
                                # it moves to the near-idle scalar engine.
                                dst = ctx_sb[:, g * E + h * HD:g * E + (h + 1) * HD]
                                if hp == NH // 2 - 1:
                                    nc.scalar.activation(
                                        dst, ps_ctx2[:, jj * HD:(jj + 1) * HD],
                                        AF.Copy, scale=rS4[:, j:j + 1])
                                else:
                                    nc.vector.tensor_scalar_mul(
                                        dst, ps_ctx2[:, jj * HD:(jj + 1) * HD],
                                        rS4[:, j:j + 1])
                        if h == NH - 1:
                            # psu column complete: z fused here so the
                            # back-to-back z mms overlap the attn pipeline
                            for j in range(4):
                                g = g0 + j
                                ind_u = attp.tile([P, GS], BF16, tag="iu")
                                nc.vector.tensor_scalar(
                                    out=ind_u[:], in0=ind_sb[:],
                                    scalar1=psu_all[:, g:g + 1],
                                    scalar2=1.0 / (L * NH),
                                    op0=mybir.AluOpType.mult,
                                    op1=mybir.AluOpType.mult)
                                for ec in range(KE):
                                    nc.tensor.matmul(
                                        zref['t'][:, ec * BC + g * GS:ec * BC + (g + 1) * GS],
                                        ctx_sb[:, g * E + ec * P:g * E + (ec + 1) * P],
                                        ind_u[:], start=True, stop=True)

                    stash = {}

                    def S(gb):
                        stash[gb] = _scores(gb)

                    def T(gb):
                        _tail(stash.pop(gb), gb)

                    return S, T

                pend = []
                for hp in range(NH // 2):
                    qku, qT_sb, kT_sb = _qk_units(hp)
                    qkT.append((qT_sb, kT_sb))
                    for i, u in enumerate(qku):
                        u()
                        if i < len(pend):
                            pend[i]()
                    for u in pend[len(qku):]:
                        u()
                    S0, T0 = _mk_head(hp, 0)
                    S1, T1 = _mk_head(hp, 1)
                    pend = [lambda: S0(0), lambda: S0(1),
                            lambda: T0(0), lambda: T0(1),
                            lambda: S1(0), lambda: S1(1),
                            lambda: T1(0), lambda: T1(1)]
                    if hp == NH // 2 - 1:
                        # last pair flushes with nothing to hide behind:
                        # round-robin S/T across both heads for deeper
                        # cross-unit pipelining. z accumulates into its own
                        # psu-pool bank.
                        zref['t'] = psu.tile([P, KE * BC], F32,
                                             name="ps_zT")
                for u in pend:
                    u()

                nc.vector.tensor_copy(zT_sb[:], zref['t'][:])

        # ---------------- MLP head (w_out folded into w1eff) ----------------
        with ExitStack() as s4:
            ps4 = s4.enter_context(tc.tile_pool(name="ps4", bufs=4, space="PSUM"))
            w1s3 = w1s[:].rearrange("p (ke n) -> p ke n", ke=KE)
            for nt in range(2):
                psum = ps4.tile([P, BC], F32, tag="mm")
                for ke in range(KE):
                    nc.tensor.matmul(psum[:], w1s3[:, ke, nt * P:(nt + 1) * P],
                                     zT_sb[:, ke * BC:(ke + 1) * BC],
                                     start=(ke == 0), stop=(ke == KE - 1))
                nc.scalar.activation(o1T_sb[:, nt * BC:(nt + 1) * BC], psum[:],
                                     AF.Relu, bias=b1T_sb[:, nt:nt + 1])
            psum = ps4.tile([P, BC], F32, tag="mm")
            w2s3 = w2_sb[:].rearrange("p (ke n) -> p ke n", ke=2)
            for ke in range(2):
                nc.tensor.matmul(psum[:], w2s3[:, ke, :],
                                 o1T_sb[:, ke * BC:(ke + 1) * BC],
                                 start=(ke == 0), stop=(ke == 1))
            nc.scalar.activation(o2T_sb[:], psum[:], AF.Relu,
                                 bias=b2T_sb[:, 0:1])
            psum3 = ps4.tile([P, BC], F32, tag="mm")
            nc.tensor.matmul(psum3[0:64, :], w3_sb[:], o2T_sb[:], start=True, stop=True)
            nc.scalar.activation(o3T_sb[:], psum3[0:64, :], AF.Relu, bias=b3T_sb[:, 0:1])
            psum4 = ps4.tile([P, BC], F32, tag="mm")
            nc.tensor.matmul(psum4[0:1, :], w4_sb[:], o3T_sb[:], start=True, stop=True)
            # sigmoid(x) = 1/(1+exp(-x)) via the resident exp table: avoids
            # a 1.3us ACT_TABLE_LOAD for AF.Sigmoid on the critical tail.
            en_sb = acts.tile([1, BC], F32)
            nc.scalar.activation(en_sb[:], psum4[0:1, :], AF.Exp,
                                 bias=b4n_sb[:, 0:1], scale=-1.0)
            nc.vector.tensor_scalar_add(en_sb[:], en_sb[:], 1.0)
            nc.vector.reciprocal(outT_sb[:], en_sb[:])
            nc.sync.dma_start(out[:, 0:1], outT_sb[0:1, :])

    nc.compile()
    return nc


_BF = ml_dtypes.bfloat16
_F8 = ml_dtypes.float8_e4m3


def _q8(a, scale):
    return np.clip(np.asarray(a, np.float32) * scale, -240.0, 240.0).astype(_F8)


def _mbl():
    m = np.zeros((GS, P), np.float32)
    for j in range(GS):
        m[j, j * L:(j + 1) * L] = 1.0
    return m.astype(_BF)


def _mbr():
    m = np.full((GS, P), -1600.0 * SQK * SQK, np.float32)
    for j in range(GS):
        m[j, j * L:(j + 1) * L] = 0.0
    return m.astype(_BF)


def _prep_shared(w_red, b_red, w_in, b_in, w_out, b_out, w1, b1, w2, b2, w3, b3,
                 w4, b4):
    f32 = np.float32
    w_red, w_in, w_out = (np.asarray(a, f32) for a in (w_red, w_in, w_out))
    w1 = np.asarray(w1, f32)
    b_in = np.asarray(b_in, f32)
    b_out_eff = np.asarray(b_out, f32) + w_out @ b_in[2 * E:3 * E]
    w1_eff = w1 @ w_out                     # (256, E)
    b1_eff = np.asarray(b1, f32) + w1 @ b_out_eff
    w8 = _q8(w_red.T, WRS)          # [H, E]
    wredS = np.ascontiguousarray(
        w8.reshape(KX, P, KE // 2, 2 * P).transpose(1, 2, 0, 3)
        .reshape(P, -1))
    win8 = _q8(w_in.T, WIS)         # [E, 3E]
    def _stripe4(block):            # [E, 2048] -> [P, 4*KE*512]
        return np.ascontiguousarray(
            block.reshape(KE, P, 4, 512).transpose(1, 2, 0, 3).reshape(P, -1))
    w1Sa = np.ascontiguousarray(
        w1_eff.T.astype(_BF).reshape(KE, P, 256).transpose(1, 0, 2)
        .reshape(P, -1))
    shared = {
        "wredS": wredS,
        "winSq": _stripe4(win8[:, :E]),
        "winSk": _stripe4(win8[:, E:2 * E]),
        "winSv": _stripe4(win8[:, 2 * E:]),
        "w1S": w1Sa,
        "w2T": np.ascontiguousarray(np.asarray(w2, f32).T).astype(_BF),
        "w3T": np.ascontiguousarray(np.asarray(w3, f32).T).astype(_BF),
        "w4T": np.ascontiguousarray(np.asarray(w4, f32).T).astype(_BF),
        "bredT16": np.ascontiguousarray(
            (np.asarray(b_red, f32) * HS).reshape(KE, P).T),
        "binT": np.ascontiguousarray((b_in[:2 * E] * SQK).reshape(2 * KE, P).T),
        "b1effT": np.ascontiguousarray(b1_eff.reshape(2, P).T),
        "b2T": np.ascontiguousarray(np.asarray(b2, f32).reshape(1, P).T),
        "b3T": np.ascontiguousarray(np.asarray(b3, f32).reshape(1, 64).T),
        "b4": np.asarray(b4, f32).reshape(1, 1),
        "mbl": _mbl(), "mbr": _mbr(),
    }
    return shared


def kernel(x, w_red, b_red, w_in, b_in, w_out, b_out, w1, b1, w2, b2, w3, b3,
           w4, b4):
    global LAST_EXEC_TIME_NS
    x = np.asarray(x, np.float32)
    shared = _prep_shared(w_red, b_red, w_in, b_in, w_out, b_out, w1, b1, w2,
                          b2, w3, b3, w4, b4)
    in_maps = []
    for c in range(NCORES):
        xc = x[c * BC:(c + 1) * BC].reshape(M, H)
        x8 = _q8(xc.T, XS)  # [H, M]
        xSc = np.ascontiguousarray(
            x8.reshape(KX, P, M).transpose(1, 0, 2).reshape(P, -1))
        in_maps.append({"xS": xSc, **shared})
    nc = _build_kernel()
    trace = os.environ.get("BASS_TRACE", "0") == "1"
    kw = {}
    if trace:
        _install_ntff_hook_shim()
        import concourse.bass_utils as _bu
        _bu.upload_artifacts = lambda d: str(d)  # no artifact bucket here
        tmpdir = os.environ.get("BASS_TRACE_DIR", "/tmp/bass_trace")
        os.makedirs(tmpdir, exist_ok=True)
        kw = {"trace": True, "tmpdir": tmpdir}
    res = run_bass_kernel_spmd(nc, in_maps, core_ids=list(range(NCORES)), **kw)
    LAST_EXEC_TIME_NS = res.exec_time_ns
    return np.concatenate([res.results[c]["out"] for c in range(NCORES)], axis=0)


if __name__ == "__main__":
    print("smoke test: building kernel only")
    _build_kernel()
    print("build OK")



# revision 51
# speedup vs baseline: 1.0076x; 1.0076x over previous
"""Trainium2 Bass kernel for AttentionMLPReduction (fp8 DoubleRow version).

Reference computation (per sample, B=256, L=32, H=4096, E=2048, NH=8, hd=256):
  h    = relu(x @ w_red.T + b_red)                  (B,L,E)
  qkv  = h @ w_in.T + b_in ; q,k,v = split(qkv)
  attn = softmax(q @ k.T / sqrt(hd))  per head      (B,NH,L,L)
  ctx  = attn @ v                                   (B,NH,L,hd) -> (B,L,E)
  attn_output = ctx @ w_out.T + b_out               (B,L,E)
  w_mean = attn.mean(heads); w_norm = w_mean / rowsum  (== w_mean)
  pooled = mean_q(w_norm @ attn_output)             (B,E)
  out = sigmoid(mlp(pooled))                        (B,1)

Algebraic simplifications:
  * w_norm == w_mean exactly (rows already sum to 1).
  * pooled[b] = u[b] @ attn_output[b] with u[b,l] = mean_q w_mean[b,q,l].
  * z[b] := u[b] @ ctx[b]; pooled = z @ w_out.T + b_out_eff with
    b_out_eff = b_out + w_out @ b_in_v (since sum_l u[b,l] == 1).
  * w_out is folded into the MLP head entirely:
      o1 = relu(pooled @ w1.T + b1) = relu(z @ (w1 @ w_out).T + b1_eff),
      b1_eff = b1 + w1 @ b_out_eff, so the (B,E)x(E,E) GEMM3 disappears.

Precision: the three big GEMMs (x@w_red, h@w_qk, h@w_v) run in fp8 e4m3 with
MatmulPerfMode.DoubleRow (2 k-rows per PE pass). Host pre-scales operands to
the e4m3 sweet range with power-of-2 factors (exact), and the dequant scales
fold into the psum->sbuf activation step:
  x*32, w_red*4096 -> h8 = 16*h via relu scale 1/8192
  h8 (=16*h), w_in*8192 -> q,k,v via scale 1/131072
Softmax block-diag masking is folded into the scores psum as a rank-4 matmul
driving off-diagonal 32x32 blocks to exp(-100) == 0. Schedule: GEMM1 (DMA
issue order tuned so x/wred sub-tiles arrive just-in-time), v GEMM, then per
head-pair qk GEMMs with the previous pair's attention interleaved 1:1 into
the fp8 streams; z is fused into the last pair's tails; the final sigmoid
avoids an ACT_TABLE_LOAD by using exp + reciprocal.

Sharding: pure data parallel over batch; 32 samples per core, weights
replicated (prepared/cast once on the host).

Per-core layouts (partition dim first):
  xT8   [H=4096, M=1024] fp8   (M = 32 samples x L=32), value = 32*x
  hT    [E=2048, M]      fp8   in SBUF, value = 16*h
  qT,kT per head-pair: [128, 4*M] fp8 (col = dtile*M + m), value = 32*q
    -> scores run as one fp8-DR matmul over both d-tiles per group
  v     [M, E]           bf16   natural orientation
  ctx   [M, E]           bf16
  zT    [E, 32]          bf16 -> o1T [256,32] -> ... -> outT [1,32]
"""

import os
import numpy as np
import ml_dtypes

import concourse.bass as bass
import concourse.mybir as mybir
import concourse.tile as tile
from concourse import bacc
from concourse.bass_utils import run_bass_kernel_spmd
from concourse.masks import make_identity

BF16 = mybir.dt.bfloat16
F8 = mybir.dt.float8e4
F32 = mybir.dt.float32
AF = mybir.ActivationFunctionType
DR = mybir.MatmulPerfMode.DoubleRow

B, L, H, E, NH = 256, 32, 4096, 2048, 8
HD = E // NH  # 256
NCORES = 8
BC = B // NCORES  # 32 samples per core
M = BC * L  # 1024 rows per core
P = 128
KX = H // P  # 32 k-tiles for GEMM1
KE = E // P  # 16 k-tiles for E-contraction GEMMs
MT = M // P  # 8 m-tiles
GS = P // L  # 4 samples per partition-tile

# fp8 scaling (all powers of two -> exact to fold/unfold)
XS = 32.0        # x pre-scale
WRS = 4096.0     # w_red pre-scale
HS = 16.0        # h post-scale (stored h8 = HS*h)
WIS = 8192.0     # w_in pre-scale
G1_SCALE = HS / (XS * WRS)      # psum -> h8
QKV_SCALE = 1.0 / (HS * WIS)    # psum -> q/k/v
SQK = 32.0   # q/k post-scale: qT/kT stored fp8 as 32*q (|q|<3.4 -> <109)

# module-level stash for the last run's HW exec time (ns), if traced
LAST_EXEC_TIME_NS = None


def _install_ntff_hook_shim():
    """antenv.axon_hooks is missing in this container; bass_utils imports it
    when trace=True under axon. Recreate it and register the ctypes-driven
    NRT profile hook from trn_boot if available."""
    import sys
    import types
    try:
        from antenv import axon_hooks  # noqa: F401
        return
    except ImportError:
        pass
    try:
        import antenv
    except ImportError:
        return
    m = types.ModuleType("antenv.axon_hooks")
    m._hook = None
    m.set_axon_ntff_profile_hook = lambda h: setattr(m, "_hook", h)
    m.get_axon_ntff_profile_hook = lambda: m._hook
    sys.modules["antenv.axon_hooks"] = m
    antenv.axon_hooks = m
    try:
        from trn_agent_boot.trn_boot import _ntff_profile_via_ctypes
        hook = _ntff_profile_via_ctypes("/opt/axon/libaxon_pjrt.so")
        if hook is not None:
            m._hook = hook
    except Exception:
        pass


def _build_kernel() -> bass.Bass:
    nc = bacc.Bacc(None, target_bir_lowering=False, debug=False)

    # ---- DRAM parameters (per-core shard views) ----
    # pre-striped on host: every row below is the exact SBUF line for one
    # partition, so DMAs are contiguous 8-16KB lines at full HBM speed.
    xS = nc.dram_tensor("xS", [P, KX * M], F8, kind="ExternalInput")
    wredS = nc.dram_tensor("wredS", [P, (KE // 2) * KX * 2 * P], F8,
                           kind="ExternalInput")
    winSq = nc.dram_tensor("winSq", [P, 4 * KE * 512], F8, kind="ExternalInput")
    winSk = nc.dram_tensor("winSk", [P, 4 * KE * 512], F8, kind="ExternalInput")
    winSv = nc.dram_tensor("winSv", [P, 4 * KE * 512], F8, kind="ExternalInput")
    w1S = nc.dram_tensor("w1S", [P, KE * 256], BF16, kind="ExternalInput")
    w2T = nc.dram_tensor("w2T", [256, P], BF16, kind="ExternalInput")
    w3T = nc.dram_tensor("w3T", [P, 64], BF16, kind="ExternalInput")
    w4T = nc.dram_tensor("w4T", [64, 1], BF16, kind="ExternalInput")
    bredT16 = nc.dram_tensor("bredT16", [P, KE], F32, kind="ExternalInput")
    binT = nc.dram_tensor("binT", [P, 2 * KE], F32, kind="ExternalInput")
    b1effT = nc.dram_tensor("b1effT", [P, 2], F32, kind="ExternalInput")
    b2T = nc.dram_tensor("b2T", [P, 1], F32, kind="ExternalInput")
    b3T = nc.dram_tensor("b3T", [64, 1], F32, kind="ExternalInput")
    b4 = nc.dram_tensor("b4", [1, 1], F32, kind="ExternalInput")
    mbl = nc.dram_tensor("mbl", [GS, P], BF16, kind="ExternalInput")
    mbr = nc.dram_tensor("mbr", [GS, P], BF16, kind="ExternalInput")
    out = nc.dram_tensor("out", [BC, 1], F32, kind="ExternalOutput")

    from contextlib import ExitStack

    with tile.TileContext(nc) as tc, ExitStack() as ctx:
        const = ctx.enter_context(tc.tile_pool(name="const", bufs=1))
        bredT_sb = const.tile([P, KE], F32)
        binT_sb = const.tile([P, 2 * KE], F32)
        b1T_sb = const.tile([P, 2], F32)
        b2T_sb = const.tile([P, 1], F32)
        b3T_sb = const.tile([64, 1], F32)
        b4_sb = const.tile([1, 1], F32)
        b4n_sb = const.tile([1, 1], F32)
        ind_sb = const.tile([P, GS], BF16)
        ident_sb = const.tile([P, P], BF16)
        mbl_sb = const.tile([GS, P], BF16)
        mbr_sb = const.tile([GS, P], BF16)

        def _const_dmas():
            # all consts ride the scalar ring AFTER the x chunks: the gpsimd
            # software ring only moves ~16KB/25us once the kernel is running
            # and its final trickle blocked kernel teardown by ~4us.
            nc.scalar.dma_start(binT_sb[:], binT[:])
            nc.scalar.dma_start(b1T_sb[:], b1effT[:])
            nc.scalar.dma_start(b2T_sb[:], b2T[:])
            nc.scalar.dma_start(b3T_sb[:], b3T[:])
            nc.scalar.dma_start(b4_sb[:], b4[:])
            nc.vector.tensor_scalar_mul(b4n_sb[:], b4_sb[:], -1.0)
            # indicator[p, j] = 1.0 if p // 32 == j else 0 (for z block-sums)
            nc.any.memset(ind_sb[:], 0.0)
            for j in range(GS):
                nc.any.memset(ind_sb[j * L:(j + 1) * L, j:j + 1], 1.0)
            make_identity(nc, ident_sb)
            # rank-4 factors of the block-diag mask bias: -1600*SQK^2 off-
            # diagonal, 0 on-diagonal (scores psum += mbl.T @ mbr).
            nc.scalar.dma_start(mbl_sb[:], mbl[:])
            nc.scalar.dma_start(mbr_sb[:], mbr[:])

        # persistent activations (live across phases)
        acts = ctx.enter_context(tc.tile_pool(name="acts", bufs=1))
        hT_sb = acts.tile([P, KE * M], F8)         # col = et*M + m, = 16*h
        zT_sb = acts.tile([P, KE * BC], BF16)      # col = ec*BC + s
        o1T_sb = acts.tile([P, 2 * BC], BF16)
        o2T_sb = acts.tile([P, BC], BF16)
        o3T_sb = acts.tile([64, BC], BF16)
        outT_sb = acts.tile([1, BC], F32)

        # x + weights streamed early. Scalar ring (Q10): xT, v stripes, w1
        # family, then ALL qk pair stripes (issued after GEMM1 so they never
        # queue ahead of wred stripes). Sync ring (Q1): GEMM1 eg stripes
        # only, bufs=3 with issue-ahead so eg arrivals always lead compute.
        # Gpsimd ring starts earliest (~5us): it carries the first-needed
        # x/wred sub-tiles so the first matmul can fire ~7us in.
        w1p = ctx.enter_context(tc.tile_pool(name="w1p", bufs=1))
        w1s = w1p.tile([P, KE * 256], BF16)
        w2_sb = w1p.tile([P, 2 * P], BF16)
        w3_sb = w1p.tile([P, 64], BF16)
        w4_sb = w1p.tile([64, 1], BF16)
        EG = 2  # e-tiles per wred stripe

        def _eg_dma(eg):
            stripe = wpool.tile([P, KX * EG * P], F8, name=f"wrs{eg}",
                                tag="wrs")
            sz = KX * EG * P
            nc.sync.dma_start(stripe[:], wredS[:, eg * sz:(eg + 1) * sz])
            return stripe

        wqk = ctx.enter_context(tc.tile_pool(name="winqk", bufs=2))
        vpool = ctx.enter_context(tc.tile_pool(name="vctx", bufs=1))
        v_sb = vpool.tile([P, MT * E], BF16)    # col = mt*E + f
        ctx_sb = vpool.tile([P, MT * E], BF16)  # col = mt*E + e
        wv_cm = tc.tile_pool(name="winv", bufs=1)
        wv = wv_cm.__enter__()
        v_stripes = [wv.tile([P, KE * 512], F8, name=f"vst{fc}", tag=f"vst{fc}")
                     for fc in range(4)]
        # wred + x pools are innermost so both can close (LIFO) right after
        # GEMM1, freeing 48KB/partition for the attention-phase pools.
        wpool_cm = tc.tile_pool(name="wred", bufs=2)
        wpool = wpool_cm.__enter__()
        xpool_cm = tc.tile_pool(name="xT", bufs=1)
        xpool = xpool_cm.__enter__()
        xq = [xpool.tile([P, (KX // 4) * M], F8, name=f"xq{i}", tag=f"xq{i}")
              for i in range(4)]

        # --- scalar ring: bredT first (needed ~25us), then x in 0.5MB
        # halves so GEMM1's q-quarters never wait on a full 1MB chunk, then
        # the MLP tail weights (arrive ~25us; needed ~350us).
        QSZ = KE * 512
        XSZ = (KX // 4) * M
        for i in range(4):
            # kt-pair quarters for the first two chunks (finer arrival
            # granularity while DMA is oversubscribed), halves after.
            nsub = 4 if i < 2 else 2
            w = XSZ // nsub
            for hf in range(nsub):
                c0 = i * XSZ + hf * w
                nc.scalar.dma_start(xq[i][:, hf * w:(hf + 1) * w],
                                    xS[:, c0:c0 + w])
        nc.scalar.dma_start(
            w2_sb[:].rearrange("p (ke n) -> p ke n", ke=2),
            w2T[:].rearrange("(ke p) n -> p ke n", p=P))
        nc.scalar.dma_start(w3_sb[:], w3T[:])
        nc.scalar.dma_start(w4_sb[:], w4T[:])
        _const_dmas()
        # --- sync ring: wred ONLY during GEMM1 (q0 quarter first so the
        # first matmuls wait on 256KB, not 1MB); v/w1 queue BEHIND the eg
        # stripes (emitted after the GEMM1 loop) so they can never starve
        # the wred stream mid-GEMM.
        wrs0 = wpool.tile([P, KX * EG * P], F8, name="wrs0", tag="wrs")
        kw = 8 * EG * P  # wred cols for kt0-7 (the q=0 quarter)
        nc.sync.dma_start(wrs0[:, 0:kw], wredS[:, 0:kw])
        nc.sync.dma_start(wrs0[:, kw:KX * EG * P], wredS[:, kw:KX * EG * P])
        # bredT (64B rows, packet-slow) hides behind wrs0 on the sync ring,
        # done ~14us, needed ~30us.
        nc.sync.dma_start(bredT_sb[:], bredT16[:])
        eg_stripes = {0: wrs0, 1: _eg_dma(1)}
        qk_stripes = {}  # (hp, 'qs'/'ks') -> tile; filled after GEMM1

        with ExitStack() as s1:
            xq3 = [t[:].rearrange("p (kt m) -> p kt m", kt=KX // 4) for t in xq]
            ps1 = s1.enter_context(tc.tile_pool(name="ps1", bufs=2, space="PSUM"))
            for eg in range(KE // EG):
                stripe = eg_stripes[eg]
                if eg + 2 < KE // EG:
                    eg_stripes[eg + 2] = _eg_dma(eg + 2)
                w3r = stripe[:].rearrange("p (kt e) -> p kt e", kt=KX)
                # 4 open psum groups (el, mc), accumulated in 4 quarter-k
                # passes so compute can start after the first x chunk lands.
                psums = {}
                for el in range(EG):
                    for mc in range(2):
                        psums[el, mc] = ps1.tile(
                            [P, 512], F32, name=f"g1ps{el}{mc}",
                            tag=f"g1ps{el}{mc}")
                for q in range(4):
                    for el in range(EG):
                        for mc in range(2):
                            for kp in range(4):
                                nc.tensor.matmul(
                                    psums[el, mc][:],
                                    w3r[:, q * 8 + 2 * kp:q * 8 + 2 * kp + 2,
                                        el * P:(el + 1) * P],
                                    xq3[q][:, 2 * kp:2 * kp + 2,
                                           mc * 512:(mc + 1) * 512],
                                    start=(q == 0 and kp == 0),
                                    stop=(q == 3 and kp == 3),
                                    perf_mode=DR)
                for el in range(EG):
                    et = eg * EG + el
                    for mc in range(2):
                        nc.scalar.activation(
                            hT_sb[:, et * M + mc * 512:et * M + (mc + 1) * 512],
                            psums[el, mc][:], AF.Relu,
                            bias=bredT_sb[:, et:et + 1], scale=G1_SCALE)
        xpool_cm.__exit__(None, None, None)  # xq dead after GEMM1
        wpool_cm.__exit__(None, None, None)  # wred stripes dead after GEMM1
        # v stripes + w1 ride the sync ring BEHIND the eg stripes (arrive
        # ~110us, needed at the v GEMM ~125us).
        for fc in range(4):
            nc.sync.dma_start(v_stripes[fc][:],
                              winSv[:, fc * QSZ:(fc + 1) * QSZ])
        nc.sync.dma_start(w1s[:], w1S[:])
        # qk pair-0/1 stripes on the now-idle scalar ring (arrive ~40us)
        for hp in range(2):
            for tag, src in (("qs", winSq), ("ks", winSk)):
                st = wqk.tile([P, KE * 512], F8, name=f"{tag}{hp}", tag=tag)
                nc.scalar.dma_start(st[:], src[:, hp * QSZ:(hp + 1) * QSZ])
                qk_stripes[(hp, tag)] = st
        h3 = hT_sb[:].rearrange("p (ke m) -> p ke m", ke=KE)

        # -------- qk GEMMs (all 4 pairs), then v GEMM + ALL attention ------
        # Attention needs v only as ctx-matmul rhs, so the entire attention
        # pipeline (scores/exp/softmax/u/ctx) interleaves into the v GEMM's
        # fp8 stream; z matmuls are deferred to one dense batch at the end
        # (back-to-back z mms pipeline at ~26ns each).
        with ExitStack() as s2:
            # ---- v GEMM first (attention ctx needs v); its psum pool
            # closes before the attention pools open.
            def _v_unit(psv, mt, fc):
                ps = psv.tile([P, 512], F32, name="vps", tag="vps")
                st3 = v_stripes[fc][:].rearrange("p (ke f) -> p ke f", ke=KE)
                for kp in range(KE // 2):
                    nc.tensor.matmul(
                        ps[:],
                        h3[:, 2 * kp:2 * kp + 2, mt * P:(mt + 1) * P],
                        st3[:, 2 * kp:2 * kp + 2, :],
                        start=(kp == 0), stop=(kp == KE // 2 - 1),
                        perf_mode=DR)
                dst = v_sb[:, mt * E + fc * 512:mt * E + (fc + 1) * 512]
                if fc % 2 == 0:
                    nc.scalar.activation(dst, ps[:], AF.Copy,
                                         scale=QKV_SCALE)
                else:
                    nc.vector.tensor_scalar_mul(dst, ps[:], QKV_SCALE)

            with ExitStack() as s2v:
                psv = s2v.enter_context(
                    tc.tile_pool(name="psv", bufs=4, space="PSUM"))
                for mt in range(MT):
                    for fc in range(4):
                        _v_unit(psv, mt, fc)
            wv_cm.__exit__(None, None, None)  # v stripes dead after v GEMM

            with ExitStack() as s2b:
                qk_out = s2b.enter_context(tc.tile_pool(name="qkT", bufs=4))
                psqk = s2b.enter_context(tc.tile_pool(name="psqk", bufs=2, space="PSUM"))
                psc = s2b.enter_context(tc.tile_pool(name="psc", bufs=2, space="PSUM"))
                psu = s2b.enter_context(tc.tile_pool(name="psu", bufs=1, space="PSUM"))
                attp = s2b.enter_context(tc.tile_pool(name="attp", bufs=4))
                atq = s2b.enter_context(tc.tile_pool(name="atq", bufs=3))
                # u accumulated across all heads in one psum bank; column
                # g's accumulation group spans h=0..7.
                psu_all = psu.tile([P, MT], F32, name="psu_all")

                def _qk_units(hp):
                    # 8 emission units (2 dst x 4 dl) for head-pair hp
                    if hp < 2:
                        q_stripe = qk_stripes[(hp, "qs")]
                        k_stripe = qk_stripes[(hp, "ks")]
                    else:
                        q_stripe = wqk.tile([P, KE * 512], F8, tag="qs")
                        k_stripe = wqk.tile([P, KE * 512], F8, tag="ks")
                        nc.scalar.dma_start(q_stripe[:],
                                            winSq[:, hp * QSZ:(hp + 1) * QSZ])
                        nc.scalar.dma_start(k_stripe[:],
                                            winSk[:, hp * QSZ:(hp + 1) * QSZ])
                    # qT2/kT2: col = dl*M + m, dl 0..3 (dtile = 4*hp + dl)
                    # stored fp8 (= SQK*q): scores run as ONE DoubleRow
                    # matmul per group, and all 4 pairs fit in SBUF.
                    qT_sb = qk_out.tile([P, 4 * M], F8, tag="qT")
                    kT_sb = qk_out.tile([P, 4 * M], F8, tag="kT")
                    units = []
                    for dst, stripe, bcol0 in ((qT_sb, q_stripe, 4 * hp),
                                               (kT_sb, k_stripe, KE + 4 * hp)):
                        s3 = stripe[:].rearrange("p (ke f) -> p ke f", ke=KE)
                        for dl in range(4):
                            def _u(dst=dst, s3=s3, bcol0=bcol0, dl=dl):
                                psums = [psqk.tile([P, 512], F32,
                                                   name=f"qkps{i}",
                                                   tag=f"qkps{i}")
                                         for i in range(2)]
                                for kp in range(KE // 2):
                                    for mc in range(2):
                                        nc.tensor.matmul(
                                            psums[mc][:],
                                            s3[:, 2 * kp:2 * kp + 2, dl * P:(dl + 1) * P],
                                            h3[:, 2 * kp:2 * kp + 2, mc * 512:(mc + 1) * 512],
                                            start=(kp == 0),
                                            stop=(kp == KE // 2 - 1),
                                            perf_mode=DR)
                                for mc in range(2):
                                    d_ap = dst[:, dl * M + mc * 512:dl * M + (mc + 1) * 512]
                                    if mc == 0:
                                        nc.scalar.activation(
                                            d_ap, psums[mc][:], AF.Identity,
                                            bias=binT_sb[:, bcol0 + dl:bcol0 + dl + 1],
                                            scale=QKV_SCALE * SQK)
                                    else:
                                        nc.vector.tensor_scalar(
                                            out=d_ap, in0=psums[mc][:],
                                            scalar1=QKV_SCALE * SQK,
                                            scalar2=binT_sb[:, bcol0 + dl:bcol0 + dl + 1],
                                            op0=mybir.AluOpType.mult,
                                            op1=mybir.AluOpType.add)
                            units.append(_u)
                    return units, qT_sb, kT_sb

                qkT = []

                # ---- v GEMM units: (mt, fc) granularity, fc-outer so only
                # one psum group is open at a time (2 banks via rotation).
                # ---- attention S/T closures per (head, group-batch) ----
                zref = {}

                def _mk_head(hp, hh):
                    h = 2 * hp + hh
                    d0 = 2 * hh
                    qT_sb, kT_sb = qkT[hp]
                    qv3 = qT_sb[:].rearrange("p (dl m) -> p dl m", dl=4)
                    kv3 = kT_sb[:].rearrange("p (dl m) -> p dl m", dl=4)

                    def _scores(gb):
                        g0 = gb * 4
                        ps_s4 = psqk.tile([P, 4 * P], F32,
                                          name=f"s4_{h}_{gb}", tag="qkps0")
                        for j in range(4):
                            g = g0 + j
                            sl = ps_s4[:, j * P:(j + 1) * P]
                            nc.tensor.matmul(
                                sl,
                                qv3[:, d0:d0 + 2, g * P:(g + 1) * P],
                                kv3[:, d0:d0 + 2, g * P:(g + 1) * P],
                                start=True, stop=False, perf_mode=DR)
                            nc.tensor.matmul(sl, mbl_sb[:], mbr_sb[:],
                                             start=False, stop=True)
                        # bd4 = block-diag masked exp(scores/sqrt(hd))
                        bd4 = attp.tile([P, 4 * P], BF16, tag="bd4")
                        nc.scalar.activation(
                            bd4[:], ps_s4[:], AF.Exp,
                            scale=1.0 / (np.sqrt(HD) * SQK * SQK))
                        S4 = attp.tile([P, 4], F32, tag="S4")
                        nc.vector.tensor_reduce(
                            S4[:],
                            bd4[:].rearrange("p (j q) -> p j q", j=4),
                            axis=mybir.AxisListType.X,
                            op=mybir.AluOpType.add)
                        rS4 = attp.tile([P, 4], F32, tag="rS4")
                        nc.vector.reciprocal(rS4[:], S4[:])
                        rS4b = attp.tile([P, 4], BF16, tag="rS4b")
                        nc.vector.tensor_copy(rS4b[:], rS4[:])
                        return bd4, rS4, rS4b

                    def _tail(st, gb):
                        bd4, rS4, rS4b = st
                        g0 = gb * 4
                        bdT4_ps = psqk.tile([P, 4 * P], BF16,
                                            name=f"bdT4_{h}_{gb}",
                                            tag="qkps1")
                        for j in range(4):
                            nc.tensor.transpose(
                                bdT4_ps[:, j * P:(j + 1) * P],
                                bd4[:, j * P:(j + 1) * P], ident_sb[:])
                        bdT4 = atq.tile([P, 4 * P], BF16, tag="bdT_sb")
                        nc.vector.tensor_copy(bdT4[:], bdT4_ps[:])
                        # u matmuls fill the bdT copy latency
                        for j in range(4):
                            nc.tensor.matmul(
                                psu_all[:, g0 + j:g0 + j + 1],
                                bd4[:, j * P:(j + 1) * P],
                                rS4b[:, j:j + 1],
                                start=(h == 0), stop=(h == NH - 1))
                        for jp in range(2):
                            ps_ctx2 = psc.tile([P, 2 * HD], F32, tag="ctx")
                            for jj in range(2):
                                j = jp * 2 + jj
                                g = g0 + j
                                nc.tensor.matmul(
                                    ps_ctx2[:, jj * HD:(jj + 1) * HD],
                                    bdT4[:, j * P:(j + 1) * P],
                                    v_sb[:, g * E + h * HD:g * E + (h + 1) * HD],
                                    start=True, stop=True)
                            for jj in range(2):
                                j = jp * 2 + jj
                                g = g0 + j
                                # normalized ctx copy+scale. During the sp
                                # phases scalar is loaded with qk acts, so
                                # it rides vector; in the exposed hp3 flush
                                # vector IS the bottleneck (its in-order
                                # queue stalls the next head's softmax), so
                                # it moves to the near-idle scalar engine.
                                dst = ctx_sb[:, g * E + h * HD:g * E + (h + 1) * HD]
                                if hp == NH // 2 - 1:
                                    nc.scalar.activation(
                                        dst, ps_ctx2[:, jj * HD:(jj + 1) * HD],
                                        AF.Copy, scale=rS4[:, j:j + 1])
                                else:
                                    nc.vector.tensor_scalar_mul(
                                        dst, ps_ctx2[:, jj * HD:(jj + 1) * HD],
                                        rS4[:, j:j + 1])
                        if h == NH - 1:
                            # psu column complete: z fused here so the
                            # back-to-back z mms overlap the attn pipeline
                            for j in range(4):
                                g = g0 + j
                                ind_u = attp.tile([P, GS], BF16, tag="iu")
                                nc.vector.tensor_scalar(
                                    out=ind_u[:], in0=ind_sb[:],
                                    scalar1=psu_all[:, g:g + 1],
                                    scalar2=1.0 / (L * NH),
                                    op0=mybir.AluOpType.mult,
                                    op1=mybir.AluOpType.mult)
                                for ec in range(KE):
                                    nc.tensor.matmul(
                                        zref['t'][:, ec * BC + g * GS:ec * BC + (g + 1) * GS],
                                        ctx_sb[:, g * E + ec * P:g * E + (ec + 1) * P],
                                        ind_u[:], start=True, stop=True)

                    stash = {}

                    def S(gb):
                        stash[gb] = _scores(gb)

                    def T(gb):
                        _tail(stash.pop(gb), gb)

                    return S, T

                pend = []
                for hp in range(NH // 2):
                    qku, qT_sb, kT_sb = _qk_units(hp)
                    qkT.append((qT_sb, kT_sb))
                    for i, u in enumerate(qku):
                        u()
                        if i < len(pend):
                            pend[i]()
                    for u in pend[len(qku):]:
                        u()
                    S0, T0 = _mk_head(hp, 0)
                    S1, T1 = _mk_head(hp, 1)
                    pend = [lambda: S0(0), lambda: S0(1),
                            lambda: T0(0), lambda: T0(1),
                            lambda: S1(0), lambda: S1(1),
                            lambda: T1(0), lambda: T1(1)]
                    if hp == NH // 2 - 1:
                        # last pair flushes with nothing to hide behind:
                        # round-robin S/T across both heads for deeper
                        # cross-unit pipelining. z accumulates into its own
                        # psu-pool bank.
                        zref['t'] = psu.tile([P, KE * BC], F32,
                                             name="ps_zT")
                for u in pend:
                    u()

                nc.vector.tensor_copy(zT_sb[:], zref['t'][:])

        # ---------------- MLP head (w_out folded into w1eff) ----------------
        with ExitStack() as s4:
            ps4 = s4.enter_context(tc.tile_pool(name="ps4", bufs=4, space="PSUM"))
            w1s3 = w1s[:].rearrange("p (ke n) -> p ke n", ke=KE)
            for nt in range(2):
                psum = ps4.tile([P, BC], F32, tag="mm")
                for ke in range(KE):
                    nc.tensor.matmul(psum[:], w1s3[:, ke, nt * P:(nt + 1) * P],
                                     zT_sb[:, ke * BC:(ke + 1) * BC],
                                     start=(ke == 0), stop=(ke == KE - 1))
                nc.scalar.activation(o1T_sb[:, nt * BC:(nt + 1) * BC], psum[:],
                                     AF.Relu, bias=b1T_sb[:, nt:nt + 1])
            psum = ps4.tile([P, BC], F32, tag="mm")
            w2s3 = w2_sb[:].rearrange("p (ke n) -> p ke n", ke=2)
            for ke in range(2):
                nc.tensor.matmul(psum[:], w2s3[:, ke, :],
                                 o1T_sb[:, ke * BC:(ke + 1) * BC],
                                 start=(ke == 0), stop=(ke == 1))
            nc.scalar.activation(o2T_sb[:], psum[:], AF.Relu,
                                 bias=b2T_sb[:, 0:1])
            psum3 = ps4.tile([P, BC], F32, tag="mm")
            nc.tensor.matmul(psum3[0:64, :], w3_sb[:], o2T_sb[:], start=True, stop=True)
            nc.scalar.activation(o3T_sb[:], psum3[0:64, :], AF.Relu, bias=b3T_sb[:, 0:1])
            psum4 = ps4.tile([P, BC], F32, tag="mm")
            nc.tensor.matmul(psum4[0:1, :], w4_sb[:], o3T_sb[:], start=True, stop=True)
            # sigmoid(x) = 1/(1+exp(-x)) via the resident exp table: avoids
            # a 1.3us ACT_TABLE_LOAD for AF.Sigmoid on the critical tail.
            en_sb = acts.tile([1, BC], F32)
            nc.scalar.activation(en_sb[:], psum4[0:1, :], AF.Exp,
                                 bias=b4n_sb[:, 0:1], scale=-1.0)
            nc.vector.tensor_scalar_add(en_sb[:], en_sb[:], 1.0)
            nc.vector.reciprocal(outT_sb[:], en_sb[:])
            nc.sync.dma_start(out[:, 0:1], outT_sb[0:1, :])

    nc.compile()
    return nc


_BF = ml_dtypes.bfloat16
_F8 = ml_dtypes.float8_e4m3


def _q8(a, scale):
    return np.clip(np.asarray(a, np.float32) * scale, -240.0, 240.0).astype(_F8)


def _mbl():
    m = np.zeros((GS, P), np.float32)
    for j in range(GS):
        m[j, j * L:(j + 1) * L] = 1.0
    return m.astype(_BF)


def _mbr():
    m = np.full((GS, P), -1600.0 * SQK * SQK, np.float32)
    for j in range(GS):
        m[j, j * L:(j + 1) * L] = 0.0
    return m.astype(_BF)


def _prep_shared(w_red, b_red, w_in, b_in, w_out, b_out, w1, b1, w2, b2, w3, b3,
                 w4, b4):
    f32 = np.float32
    w_red, w_in, w_out = (np.asarray(a, f32) for a in (w_red, w_in, w_out))
    w1 = np.asarray(w1, f32)
    b_in = np.asarray(b_in, f32)
    b_out_eff = np.asarray(b_out, f32) + w_out @ b_in[2 * E:3 * E]
    w1_eff = w1 @ w_out                     # (256, E)
    b1_eff = np.asarray(b1, f32) + w1 @ b_out_eff
    w8 = _q8(w_red.T, WRS)          # [H, E]
    wredS = np.ascontiguousarray(
        w8.reshape(KX, P, KE // 2, 2 * P).transpose(1, 2, 0, 3)
        .reshape(P, -1))
    win8 = _q8(w_in.T, WIS)         # [E, 3E]
    def _stripe4(block):            # [E, 2048] -> [P, 4*KE*512]
        return np.ascontiguousarray(
            block.reshape(KE, P, 4, 512).transpose(1, 2, 0, 3).reshape(P, -1))
    w1Sa = np.ascontiguousarray(
        w1_eff.T.astype(_BF).reshape(KE, P, 256).transpose(1, 0, 2)
        .reshape(P, -1))
    shared = {
        "wredS": wredS,
        "winSq": _stripe4(win8[:, :E]),
        "winSk": _stripe4(win8[:, E:2 * E]),
        "winSv": _stripe4(win8[:, 2 * E:]),
        "w1S": w1Sa,
        "w2T": np.ascontiguousarray(np.asarray(w2, f32).T).astype(_BF),
        "w3T": np.ascontiguousarray(np.asarray(w3, f32).T).astype(_BF),
        "w4T": np.ascontiguousarray(np.asarray(w4, f32).T).astype(_BF),
        "bredT16": np.ascontiguousarray(
            (np.asarray(b_red, f32) * HS).reshape(KE, P).T),
        "binT": np.ascontiguousarray((b_in[:2 * E] * SQK).reshape(2 * KE, P).T),
        "b1effT": np.ascontiguousarray(b1_eff.reshape(2, P).T),
        "b2T": np.ascontiguousarray(np.asarray(b2, f32).reshape(1, P).T),
        "b3T": np.ascontiguousarray(np.asarray(b3, f32).reshape(1, 64).T),
        "b4": np.asarray(b4, f32).reshape(1, 1),
        "mbl": _mbl(), "mbr": _mbr(),
    }
    return shared


def kernel(x, w_red, b_red, w_in, b_in, w_out, b_out, w1, b1, w2, b2, w3, b3,
           w4, b4):
    global LAST_EXEC_TIME_NS
    x = np.asarray(x, np.float32)
    shared = _prep_shared(w_red, b_red, w_in, b_in, w_out, b_out, w1, b1, w2,
                          b2, w3, b3, w4, b4)
    in_maps = []
    for c in range(NCORES):
        xc = x[c * BC:(c + 1) * BC].reshape(M, H)
        x8 = _q8(xc.T, XS)  # [H, M]
        xSc = np.ascontiguousarray(
            x8.reshape(KX, P, M).transpose(1, 0, 2).reshape(P, -1))
        in_maps.append({"xS": xSc, **shared})
    nc = _build_kernel()
    trace = os.environ.get("BASS_TRACE", "0") == "1"
    kw = {}
    if trace:
        _install_ntff_hook_shim()
        import concourse.bass_utils as _bu
        _bu.upload_artifacts = lambda d: str(d)  # no artifact bucket here
        tmpdir = os.environ.get("BASS_TRACE_DIR", "/tmp/bass_trace")
        os.makedirs(tmpdir, exist_ok=True)
        kw = {"trace": True, "tmpdir": tmpdir}
    res = run_bass_kernel_spmd(nc, in_maps, core_ids=list(range(NCORES)), **kw)
    LAST_EXEC_TIME_NS = res.exec_time_ns
    return np.concatenate([res.results[c]["out"] for c in range(NCORES)], axis=0)


if __name__ == "__main__":
    print("smoke test: building kernel only")
    _build_kernel()
    print("build OK")



# revision 52
# speedup vs baseline: 1.0141x; 1.0065x over previous
"""Trainium2 Bass kernel for AttentionMLPReduction (fp8 DoubleRow version).

Reference computation (per sample, B=256, L=32, H=4096, E=2048, NH=8, hd=256):
  h    = relu(x @ w_red.T + b_red)                  (B,L,E)
  qkv  = h @ w_in.T + b_in ; q,k,v = split(qkv)
  attn = softmax(q @ k.T / sqrt(hd))  per head      (B,NH,L,L)
  ctx  = attn @ v                                   (B,NH,L,hd) -> (B,L,E)
  attn_output = ctx @ w_out.T + b_out               (B,L,E)
  w_mean = attn.mean(heads); w_norm = w_mean / rowsum  (== w_mean)
  pooled = mean_q(w_norm @ attn_output)             (B,E)
  out = sigmoid(mlp(pooled))                        (B,1)

Algebraic simplifications:
  * w_norm == w_mean exactly (rows already sum to 1).
  * pooled[b] = u[b] @ attn_output[b] with u[b,l] = mean_q w_mean[b,q,l].
  * z[b] := u[b] @ ctx[b]; pooled = z @ w_out.T + b_out_eff with
    b_out_eff = b_out + w_out @ b_in_v (since sum_l u[b,l] == 1).
  * w_out is folded into the MLP head entirely:
      o1 = relu(pooled @ w1.T + b1) = relu(z @ (w1 @ w_out).T + b1_eff),
      b1_eff = b1 + w1 @ b_out_eff, so the (B,E)x(E,E) GEMM3 disappears.

Precision: the three big GEMMs (x@w_red, h@w_qk, h@w_v) run in fp8 e4m3 with
MatmulPerfMode.DoubleRow (2 k-rows per PE pass). Host pre-scales operands to
the e4m3 sweet range with power-of-2 factors (exact), and the dequant scales
fold into the psum->sbuf activation step:
  x*32, w_red*4096 -> h8 = 16*h via relu scale 1/8192
  h8 (=16*h), w_in*8192 -> q,k,v via scale 1/131072
Softmax block-diag masking is folded into the scores psum as a rank-4 matmul
driving off-diagonal 32x32 blocks to exp(-100) == 0. Schedule: GEMM1 (DMA
issue order tuned so x/wred sub-tiles arrive just-in-time), v GEMM, then per
head-pair qk GEMMs with the previous pair's attention interleaved 1:1 into
the fp8 streams; z is fused into the last pair's tails; the final sigmoid
avoids an ACT_TABLE_LOAD by using exp + reciprocal.

Sharding: pure data parallel over batch; 32 samples per core, weights
replicated (prepared/cast once on the host).

Per-core layouts (partition dim first):
  xT8   [H=4096, M=1024] fp8   (M = 32 samples x L=32), value = 32*x
  hT    [E=2048, M]      fp8   in SBUF, value = 16*h
  qT,kT per head-pair: [128, 4*M] fp8 (col = dtile*M + m), value = 32*q
    -> scores run as one fp8-DR matmul over both d-tiles per group
  v     [M, E]           bf16   natural orientation
  ctx   [M, E]           bf16
  zT    [E, 32]          bf16 -> o1T [256,32] -> ... -> outT [1,32]
"""

import os
import numpy as np
import ml_dtypes

import concourse.bass as bass
import concourse.mybir as mybir
import concourse.tile as tile
from concourse import bacc
from concourse.bass_utils import run_bass_kernel_spmd
from concourse.masks import make_identity

BF16 = mybir.dt.bfloat16
F8 = mybir.dt.float8e4
F32 = mybir.dt.float32
AF = mybir.ActivationFunctionType
DR = mybir.MatmulPerfMode.DoubleRow

B, L, H, E, NH = 256, 32, 4096, 2048, 8
HD = E // NH  # 256
NCORES = 8
BC = B // NCORES  # 32 samples per core
M = BC * L  # 1024 rows per core
P = 128
KX = H // P  # 32 k-tiles for GEMM1
KE = E // P  # 16 k-tiles for E-contraction GEMMs
MT = M // P  # 8 m-tiles
GS = P // L  # 4 samples per partition-tile

# fp8 scaling (all powers of two -> exact to fold/unfold)
XS = 32.0        # x pre-scale
WRS = 4096.0     # w_red pre-scale
HS = 16.0        # h post-scale (stored h8 = HS*h)
WIS = 8192.0     # w_in pre-scale
G1_SCALE = HS / (XS * WRS)      # psum -> h8
QKV_SCALE = 1.0 / (HS * WIS)    # psum -> q/k/v
SQK = 32.0   # q/k post-scale: qT/kT stored fp8 as 32*q (|q|<3.4 -> <109)

# module-level stash for the last run's HW exec time (ns), if traced
LAST_EXEC_TIME_NS = None


def _install_ntff_hook_shim():
    """antenv.axon_hooks is missing in this container; bass_utils imports it
    when trace=True under axon. Recreate it and register the ctypes-driven
    NRT profile hook from trn_boot if available."""
    import sys
    import types
    try:
        from antenv import axon_hooks  # noqa: F401
        return
    except ImportError:
        pass
    try:
        import antenv
    except ImportError:
        return
    m = types.ModuleType("antenv.axon_hooks")
    m._hook = None
    m.set_axon_ntff_profile_hook = lambda h: setattr(m, "_hook", h)
    m.get_axon_ntff_profile_hook = lambda: m._hook
    sys.modules["antenv.axon_hooks"] = m
    antenv.axon_hooks = m
    try:
        from trn_agent_boot.trn_boot import _ntff_profile_via_ctypes
        hook = _ntff_profile_via_ctypes("/opt/axon/libaxon_pjrt.so")
        if hook is not None:
            m._hook = hook
    except Exception:
        pass


def _build_kernel() -> bass.Bass:
    nc = bacc.Bacc(None, target_bir_lowering=False, debug=False)

    # ---- DRAM parameters (per-core shard views) ----
    # pre-striped on host: every row below is the exact SBUF line for one
    # partition, so DMAs are contiguous 8-16KB lines at full HBM speed.
    xS = nc.dram_tensor("xS", [P, KX * M], F8, kind="ExternalInput")
    wredS = nc.dram_tensor("wredS", [P, (KE // 2) * KX * 2 * P], F8,
                           kind="ExternalInput")
    winSq = nc.dram_tensor("winSq", [P, 4 * KE * 512], F8, kind="ExternalInput")
    winSk = nc.dram_tensor("winSk", [P, 4 * KE * 512], F8, kind="ExternalInput")
    winSv = nc.dram_tensor("winSv", [P, 4 * KE * 512], F8, kind="ExternalInput")
    w1S = nc.dram_tensor("w1S", [P, KE * 256], BF16, kind="ExternalInput")
    w2T = nc.dram_tensor("w2T", [256, P], BF16, kind="ExternalInput")
    w3T = nc.dram_tensor("w3T", [P, 64], BF16, kind="ExternalInput")
    w4T = nc.dram_tensor("w4T", [64, 1], BF16, kind="ExternalInput")
    bredT16 = nc.dram_tensor("bredT16", [P, KE], F32, kind="ExternalInput")
    binT = nc.dram_tensor("binT", [P, 2 * KE], F32, kind="ExternalInput")
    b1effT = nc.dram_tensor("b1effT", [P, 2], F32, kind="ExternalInput")
    b2T = nc.dram_tensor("b2T", [P, 1], F32, kind="ExternalInput")
    b3T = nc.dram_tensor("b3T", [64, 1], F32, kind="ExternalInput")
    b4 = nc.dram_tensor("b4", [1, 1], F32, kind="ExternalInput")
    mbl = nc.dram_tensor("mbl", [GS, P], BF16, kind="ExternalInput")
    mbr = nc.dram_tensor("mbr", [GS, P], BF16, kind="ExternalInput")
    out = nc.dram_tensor("out", [1, BC], F32, kind="ExternalOutput")

    from contextlib import ExitStack

    with tile.TileContext(nc) as tc, ExitStack() as ctx:
        const = ctx.enter_context(tc.tile_pool(name="const", bufs=1))
        bredT_sb = const.tile([P, KE], F32)
        binT_sb = const.tile([P, 2 * KE], F32)
        b1T_sb = const.tile([P, 2], F32)
        b2T_sb = const.tile([P, 1], F32)
        b3T_sb = const.tile([64, 1], F32)
        b4_sb = const.tile([1, 1], F32)
        b4n_sb = const.tile([1, 1], F32)
        ind_sb = const.tile([P, GS], BF16)
        ident_sb = const.tile([P, P], BF16)
        mbl_sb = const.tile([GS, P], BF16)
        mbr_sb = const.tile([GS, P], BF16)

        def _const_dmas():
            # small consts ride the gpsimd software ring (slow but off the
            # two hardware rings that feed the GEMMs; all are needed late
            # enough that its ~16KB/25us trickle arrives in time).
            nc.gpsimd.dma_start(binT_sb[:], binT[:])
            nc.gpsimd.dma_start(b1T_sb[:], b1effT[:])
            nc.gpsimd.dma_start(b2T_sb[:], b2T[:])
            nc.gpsimd.dma_start(b3T_sb[:], b3T[:])
            nc.gpsimd.dma_start(b4_sb[:], b4[:])
            nc.vector.tensor_scalar_mul(b4n_sb[:], b4_sb[:], -1.0)
            # indicator[p, j] = 1.0 if p // 32 == j else 0 (for z block-sums)
            nc.any.memset(ind_sb[:], 0.0)
            for j in range(GS):
                nc.any.memset(ind_sb[j * L:(j + 1) * L, j:j + 1], 1.0)
            make_identity(nc, ident_sb)
            # rank-4 factors of the block-diag mask bias: -1600*SQK^2 off-
            # diagonal, 0 on-diagonal (scores psum += mbl.T @ mbr).
            nc.gpsimd.dma_start(mbl_sb[:], mbl[:])
            nc.gpsimd.dma_start(mbr_sb[:], mbr[:])

        # persistent activations (live across phases)
        acts = ctx.enter_context(tc.tile_pool(name="acts", bufs=1))
        hT_sb = acts.tile([P, KE * M], F8)         # col = et*M + m, = 16*h
        zT_sb = acts.tile([P, KE * BC], BF16)      # col = ec*BC + s
        o1T_sb = acts.tile([P, 2 * BC], BF16)
        o2T_sb = acts.tile([P, BC], BF16)
        o3T_sb = acts.tile([64, BC], BF16)
        outT_sb = acts.tile([1, BC], F32)

        # x + weights streamed early. Scalar ring (Q10): xT, v stripes, w1
        # family, then ALL qk pair stripes (issued after GEMM1 so they never
        # queue ahead of wred stripes). Sync ring (Q1): GEMM1 eg stripes
        # only, bufs=3 with issue-ahead so eg arrivals always lead compute.
        # Gpsimd ring starts earliest (~5us): it carries the first-needed
        # x/wred sub-tiles so the first matmul can fire ~7us in.
        w1p = ctx.enter_context(tc.tile_pool(name="w1p", bufs=1))
        w1s = w1p.tile([P, KE * 256], BF16)
        w2_sb = w1p.tile([P, 2 * P], BF16)
        w3_sb = w1p.tile([P, 64], BF16)
        w4_sb = w1p.tile([64, 1], BF16)
        EG = 2  # e-tiles per wred stripe

        def _eg_dma(eg):
            stripe = wpool.tile([P, KX * EG * P], F8, name=f"wrs{eg}",
                                tag="wrs")
            sz = KX * EG * P
            nc.sync.dma_start(stripe[:], wredS[:, eg * sz:(eg + 1) * sz])
            return stripe

        wqk = ctx.enter_context(tc.tile_pool(name="winqk", bufs=2))
        vpool = ctx.enter_context(tc.tile_pool(name="vctx", bufs=1))
        v_sb = vpool.tile([P, MT * E], BF16)    # col = mt*E + f
        ctx_sb = vpool.tile([P, MT * E], BF16)  # col = mt*E + e
        wv_cm = tc.tile_pool(name="winv", bufs=1)
        wv = wv_cm.__enter__()
        v_stripes = [wv.tile([P, KE * 512], F8, name=f"vst{fc}", tag=f"vst{fc}")
                     for fc in range(4)]
        # wred + x pools are innermost so both can close (LIFO) right after
        # GEMM1, freeing 48KB/partition for the attention-phase pools.
        wpool_cm = tc.tile_pool(name="wred", bufs=2)
        wpool = wpool_cm.__enter__()
        xpool_cm = tc.tile_pool(name="xT", bufs=1)
        xpool = xpool_cm.__enter__()
        xq = [xpool.tile([P, (KX // 4) * M], F8, name=f"xq{i}", tag=f"xq{i}")
              for i in range(4)]

        # --- scalar ring: bredT first (needed ~25us), then x in 0.5MB
        # halves so GEMM1's q-quarters never wait on a full 1MB chunk, then
        # the MLP tail weights (arrive ~25us; needed ~350us).
        QSZ = KE * 512
        XSZ = (KX // 4) * M
        for i in range(4):
            # kt-pair quarters for the first two chunks (finer arrival
            # granularity while DMA is oversubscribed), halves after.
            nsub = 4 if i < 2 else 2
            w = XSZ // nsub
            for hf in range(nsub):
                c0 = i * XSZ + hf * w
                nc.scalar.dma_start(xq[i][:, hf * w:(hf + 1) * w],
                                    xS[:, c0:c0 + w])
        nc.scalar.dma_start(
            w2_sb[:].rearrange("p (ke n) -> p ke n", ke=2),
            w2T[:].rearrange("(ke p) n -> p ke n", p=P))
        nc.scalar.dma_start(w3_sb[:], w3T[:])
        nc.scalar.dma_start(w4_sb[:], w4T[:])
        _const_dmas()
        # --- sync ring: wred ONLY during GEMM1 (q0 quarter first so the
        # first matmuls wait on 256KB, not 1MB); v/w1 queue BEHIND the eg
        # stripes (emitted after the GEMM1 loop) so they can never starve
        # the wred stream mid-GEMM.
        wrs0 = wpool.tile([P, KX * EG * P], F8, name="wrs0", tag="wrs")
        kw = 8 * EG * P  # wred cols for kt0-7 (the q=0 quarter)
        nc.sync.dma_start(wrs0[:, 0:kw], wredS[:, 0:kw])
        nc.sync.dma_start(wrs0[:, kw:KX * EG * P], wredS[:, kw:KX * EG * P])
        # bredT (64B rows, packet-slow) hides behind wrs0 on the sync ring,
        # done ~14us, needed ~30us.
        nc.sync.dma_start(bredT_sb[:], bredT16[:])
        eg_stripes = {0: wrs0, 1: _eg_dma(1)}
        qk_stripes = {}  # (hp, 'qs'/'ks') -> tile; filled after GEMM1

        with ExitStack() as s1:
            xq3 = [t[:].rearrange("p (kt m) -> p kt m", kt=KX // 4) for t in xq]
            ps1 = s1.enter_context(tc.tile_pool(name="ps1", bufs=2, space="PSUM"))
            for eg in range(KE // EG):
                stripe = eg_stripes[eg]
                if eg + 2 < KE // EG:
                    eg_stripes[eg + 2] = _eg_dma(eg + 2)
                w3r = stripe[:].rearrange("p (kt e) -> p kt e", kt=KX)
                # 4 open psum groups (el, mc), accumulated in 4 quarter-k
                # passes so compute can start after the first x chunk lands.
                psums = {}
                for el in range(EG):
                    for mc in range(2):
                        psums[el, mc] = ps1.tile(
                            [P, 512], F32, name=f"g1ps{el}{mc}",
                            tag=f"g1ps{el}{mc}")
                for q in range(4):
                    for el in range(EG):
                        for mc in range(2):
                            for kp in range(4):
                                nc.tensor.matmul(
                                    psums[el, mc][:],
                                    w3r[:, q * 8 + 2 * kp:q * 8 + 2 * kp + 2,
                                        el * P:(el + 1) * P],
                                    xq3[q][:, 2 * kp:2 * kp + 2,
                                           mc * 512:(mc + 1) * 512],
                                    start=(q == 0 and kp == 0),
                                    stop=(q == 3 and kp == 3),
                                    perf_mode=DR)
                for el in range(EG):
                    et = eg * EG + el
                    for mc in range(2):
                        nc.scalar.activation(
                            hT_sb[:, et * M + mc * 512:et * M + (mc + 1) * 512],
                            psums[el, mc][:], AF.Relu,
                            bias=bredT_sb[:, et:et + 1], scale=G1_SCALE)
        xpool_cm.__exit__(None, None, None)  # xq dead after GEMM1
        wpool_cm.__exit__(None, None, None)  # wred stripes dead after GEMM1
        # v stripes + w1 ride the sync ring BEHIND the eg stripes (arrive
        # ~110us, needed at the v GEMM ~125us).
        for fc in range(4):
            nc.sync.dma_start(v_stripes[fc][:],
                              winSv[:, fc * QSZ:(fc + 1) * QSZ])
        nc.sync.dma_start(w1s[:], w1S[:])
        # qk pair-0/1 stripes on the now-idle scalar ring (arrive ~40us)
        for hp in range(2):
            for tag, src in (("qs", winSq), ("ks", winSk)):
                st = wqk.tile([P, KE * 512], F8, name=f"{tag}{hp}", tag=tag)
                nc.scalar.dma_start(st[:], src[:, hp * QSZ:(hp + 1) * QSZ])
                qk_stripes[(hp, tag)] = st
        h3 = hT_sb[:].rearrange("p (ke m) -> p ke m", ke=KE)

        # -------- qk GEMMs (all 4 pairs), then v GEMM + ALL attention ------
        # Attention needs v only as ctx-matmul rhs, so the entire attention
        # pipeline (scores/exp/softmax/u/ctx) interleaves into the v GEMM's
        # fp8 stream; z matmuls are deferred to one dense batch at the end
        # (back-to-back z mms pipeline at ~26ns each).
        with ExitStack() as s2:
            # ---- v GEMM first (attention ctx needs v); its psum pool
            # closes before the attention pools open.
            def _v_unit(psv, mt, fc):
                ps = psv.tile([P, 512], F32, name="vps", tag="vps")
                st3 = v_stripes[fc][:].rearrange("p (ke f) -> p ke f", ke=KE)
                for kp in range(KE // 2):
                    nc.tensor.matmul(
                        ps[:],
                        h3[:, 2 * kp:2 * kp + 2, mt * P:(mt + 1) * P],
                        st3[:, 2 * kp:2 * kp + 2, :],
                        start=(kp == 0), stop=(kp == KE // 2 - 1),
                        perf_mode=DR)
                dst = v_sb[:, mt * E + fc * 512:mt * E + (fc + 1) * 512]
                if fc % 2 == 0:
                    nc.scalar.activation(dst, ps[:], AF.Copy,
                                         scale=QKV_SCALE)
                else:
                    nc.vector.tensor_scalar_mul(dst, ps[:], QKV_SCALE)

            with ExitStack() as s2v:
                psv = s2v.enter_context(
                    tc.tile_pool(name="psv", bufs=4, space="PSUM"))
                for mt in range(MT):
                    for fc in range(4):
                        _v_unit(psv, mt, fc)
            wv_cm.__exit__(None, None, None)  # v stripes dead after v GEMM

            with ExitStack() as s2b:
                qk_out = s2b.enter_context(tc.tile_pool(name="qkT", bufs=4))
                psqk = s2b.enter_context(tc.tile_pool(name="psqk", bufs=2, space="PSUM"))
                psc = s2b.enter_context(tc.tile_pool(name="psc", bufs=2, space="PSUM"))
                psu = s2b.enter_context(tc.tile_pool(name="psu", bufs=1, space="PSUM"))
                attp = s2b.enter_context(tc.tile_pool(name="attp", bufs=4))
                atq = s2b.enter_context(tc.tile_pool(name="atq", bufs=3))
                # u accumulated across all heads in one psum bank; column
                # g's accumulation group spans h=0..7.
                psu_all = psu.tile([P, MT], F32, name="psu_all")

                def _qk_units(hp):
                    # 8 emission units (2 dst x 4 dl) for head-pair hp
                    if hp < 2:
                        q_stripe = qk_stripes[(hp, "qs")]
                        k_stripe = qk_stripes[(hp, "ks")]
                    else:
                        q_stripe = wqk.tile([P, KE * 512], F8, tag="qs")
                        k_stripe = wqk.tile([P, KE * 512], F8, tag="ks")
                        nc.scalar.dma_start(q_stripe[:],
                                            winSq[:, hp * QSZ:(hp + 1) * QSZ])
                        nc.scalar.dma_start(k_stripe[:],
                                            winSk[:, hp * QSZ:(hp + 1) * QSZ])
                    # qT2/kT2: col = dl*M + m, dl 0..3 (dtile = 4*hp + dl)
                    # stored fp8 (= SQK*q): scores run as ONE DoubleRow
                    # matmul per group, and all 4 pairs fit in SBUF.
                    qT_sb = qk_out.tile([P, 4 * M], F8, tag="qT")
                    kT_sb = qk_out.tile([P, 4 * M], F8, tag="kT")
                    units = []
                    for dst, stripe, bcol0 in ((qT_sb, q_stripe, 4 * hp),
                                               (kT_sb, k_stripe, KE + 4 * hp)):
                        s3 = stripe[:].rearrange("p (ke f) -> p ke f", ke=KE)
                        for dl in range(4):
                            def _u(dst=dst, s3=s3, bcol0=bcol0, dl=dl):
                                psums = [psqk.tile([P, 512], F32,
                                                   name=f"qkps{i}",
                                                   tag=f"qkps{i}")
                                         for i in range(2)]
                                for kp in range(KE // 2):
                                    for mc in range(2):
                                        nc.tensor.matmul(
                                            psums[mc][:],
                                            s3[:, 2 * kp:2 * kp + 2, dl * P:(dl + 1) * P],
                                            h3[:, 2 * kp:2 * kp + 2, mc * 512:(mc + 1) * 512],
                                            start=(kp == 0),
                                            stop=(kp == KE // 2 - 1),
                                            perf_mode=DR)
                                for mc in range(2):
                                    d_ap = dst[:, dl * M + mc * 512:dl * M + (mc + 1) * 512]
                                    if mc == 0:
                                        nc.scalar.activation(
                                            d_ap, psums[mc][:], AF.Identity,
                                            bias=binT_sb[:, bcol0 + dl:bcol0 + dl + 1],
                                            scale=QKV_SCALE * SQK)
                                    else:
                                        nc.vector.tensor_scalar(
                                            out=d_ap, in0=psums[mc][:],
                                            scalar1=QKV_SCALE * SQK,
                                            scalar2=binT_sb[:, bcol0 + dl:bcol0 + dl + 1],
                                            op0=mybir.AluOpType.mult,
                                            op1=mybir.AluOpType.add)
                            units.append(_u)
                    return units, qT_sb, kT_sb

                qkT = []

                # ---- v GEMM units: (mt, fc) granularity, fc-outer so only
                # one psum group is open at a time (2 banks via rotation).
                # ---- attention S/T closures per (head, group-batch) ----
                zref = {}

                def _mk_head(hp, hh):
                    h = 2 * hp + hh
                    d0 = 2 * hh
                    qT_sb, kT_sb = qkT[hp]
                    qv3 = qT_sb[:].rearrange("p (dl m) -> p dl m", dl=4)
                    kv3 = kT_sb[:].rearrange("p (dl m) -> p dl m", dl=4)

                    def _scores(gb):
                        g0 = gb * 4
                        ps_s4 = psqk.tile([P, 4 * P], F32,
                                          name=f"s4_{h}_{gb}", tag="qkps0")
                        for j in range(4):
                            g = g0 + j
                            sl = ps_s4[:, j * P:(j + 1) * P]
                            nc.tensor.matmul(
                                sl,
                                qv3[:, d0:d0 + 2, g * P:(g + 1) * P],
                                kv3[:, d0:d0 + 2, g * P:(g + 1) * P],
                                start=True, stop=False, perf_mode=DR)
                            nc.tensor.matmul(sl, mbl_sb[:], mbr_sb[:],
                                             start=False, stop=True)
                        # bd4 = block-diag masked exp(scores/sqrt(hd))
                        bd4 = attp.tile([P, 4 * P], BF16, tag="bd4")
                        nc.scalar.activation(
                            bd4[:], ps_s4[:], AF.Exp,
                            scale=1.0 / (np.sqrt(HD) * SQK * SQK))
                        S4 = attp.tile([P, 4], F32, tag="S4")
                        nc.vector.tensor_reduce(
                            S4[:],
                            bd4[:].rearrange("p (j q) -> p j q", j=4),
                            axis=mybir.AxisListType.X,
                            op=mybir.AluOpType.add)
                        rS4 = attp.tile([P, 4], F32, tag="rS4")
                        nc.vector.reciprocal(rS4[:], S4[:])
                        rS4b = attp.tile([P, 4], BF16, tag="rS4b")
                        nc.vector.tensor_copy(rS4b[:], rS4[:])
                        return bd4, rS4, rS4b

                    def _tail(st, gb):
                        bd4, rS4, rS4b = st
                        g0 = gb * 4
                        bdT4_ps = psqk.tile([P, 4 * P], BF16,
                                            name=f"bdT4_{h}_{gb}",
                                            tag="qkps1")
                        for j in range(4):
                            nc.tensor.transpose(
                                bdT4_ps[:, j * P:(j + 1) * P],
                                bd4[:, j * P:(j + 1) * P], ident_sb[:])
                        bdT4 = atq.tile([P, 4 * P], BF16, tag="bdT_sb")
                        nc.vector.tensor_copy(bdT4[:], bdT4_ps[:])
                        # u matmuls fill the bdT copy latency
                        for j in range(4):
                            nc.tensor.matmul(
                                psu_all[:, g0 + j:g0 + j + 1],
                                bd4[:, j * P:(j + 1) * P],
                                rS4b[:, j:j + 1],
                                start=(h == 0), stop=(h == NH - 1))
                        for jp in range(2):
                            ps_ctx2 = psc.tile([P, 2 * HD], F32, tag="ctx")
                            for jj in range(2):
                                j = jp * 2 + jj
                                g = g0 + j
                                nc.tensor.matmul(
                                    ps_ctx2[:, jj * HD:(jj + 1) * HD],
                                    bdT4[:, j * P:(j + 1) * P],
                                    v_sb[:, g * E + h * HD:g * E + (h + 1) * HD],
                                    start=True, stop=True)
                            for jj in range(2):
                                j = jp * 2 + jj
                                g = g0 + j
                                # normalized ctx copy+scale. During the sp
                                # phases scalar is loaded with qk acts, so
                                # it rides vector; in the exposed hp3 flush
                                # vector IS the bottleneck (its in-order
                                # queue stalls the next head's softmax), so
                                # it moves to the near-idle scalar engine.
                                dst = ctx_sb[:, g * E + h * HD:g * E + (h + 1) * HD]
                                if hp == NH // 2 - 1:
                                    nc.scalar.activation(
                                        dst, ps_ctx2[:, jj * HD:(jj + 1) * HD],
                                        AF.Copy, scale=rS4[:, j:j + 1])
                                else:
                                    nc.vector.tensor_scalar_mul(
                                        dst, ps_ctx2[:, jj * HD:(jj + 1) * HD],
                                        rS4[:, j:j + 1])
                        if h == NH - 1:
                            # psu column complete: z fused here so the
                            # back-to-back z mms overlap the attn pipeline
                            for j in range(4):
                                g = g0 + j
                                ind_u = attp.tile([P, GS], BF16, tag="iu")
                                nc.vector.tensor_scalar(
                                    out=ind_u[:], in0=ind_sb[:],
                                    scalar1=psu_all[:, g:g + 1],
                                    scalar2=1.0 / (L * NH),
                                    op0=mybir.AluOpType.mult,
                                    op1=mybir.AluOpType.mult)
                                for ec in range(KE):
                                    nc.tensor.matmul(
                                        zref['t'][:, ec * BC + g * GS:ec * BC + (g + 1) * GS],
                                        ctx_sb[:, g * E + ec * P:g * E + (ec + 1) * P],
                                        ind_u[:], start=True, stop=True)

                    stash = {}

                    def S(gb):
                        stash[gb] = _scores(gb)

                    def T(gb):
                        _tail(stash.pop(gb), gb)

                    return S, T

                pend = []
                for hp in range(NH // 2):
                    qku, qT_sb, kT_sb = _qk_units(hp)
                    qkT.append((qT_sb, kT_sb))
                    for i, u in enumerate(qku):
                        u()
                        if i < len(pend):
                            pend[i]()
                    for u in pend[len(qku):]:
                        u()
                    S0, T0 = _mk_head(hp, 0)
                    S1, T1 = _mk_head(hp, 1)
                    pend = [lambda: S0(0), lambda: S0(1),
                            lambda: T0(0), lambda: T0(1),
                            lambda: S1(0), lambda: S1(1),
                            lambda: T1(0), lambda: T1(1)]
                    if hp == NH // 2 - 1:
                        # last pair flushes with nothing to hide behind:
                        # round-robin S/T across both heads for deeper
                        # cross-unit pipelining. z accumulates into its own
                        # psu-pool bank.
                        zref['t'] = psu.tile([P, KE * BC], F32,
                                             name="ps_zT")
                for u in pend:
                    u()

                nc.vector.tensor_copy(zT_sb[:], zref['t'][:])

        # ---------------- MLP head (w_out folded into w1eff) ----------------
        with ExitStack() as s4:
            ps4 = s4.enter_context(tc.tile_pool(name="ps4", bufs=4, space="PSUM"))
            w1s3 = w1s[:].rearrange("p (ke n) -> p ke n", ke=KE)
            for nt in range(2):
                psum = ps4.tile([P, BC], F32, tag="mm")
                for ke in range(KE):
                    nc.tensor.matmul(psum[:], w1s3[:, ke, nt * P:(nt + 1) * P],
                                     zT_sb[:, ke * BC:(ke + 1) * BC],
                                     start=(ke == 0), stop=(ke == KE - 1))
                nc.scalar.activation(o1T_sb[:, nt * BC:(nt + 1) * BC], psum[:],
                                     AF.Relu, bias=b1T_sb[:, nt:nt + 1])
            psum = ps4.tile([P, BC], F32, tag="mm")
            w2s3 = w2_sb[:].rearrange("p (ke n) -> p ke n", ke=2)
            for ke in range(2):
                nc.tensor.matmul(psum[:], w2s3[:, ke, :],
                                 o1T_sb[:, ke * BC:(ke + 1) * BC],
                                 start=(ke == 0), stop=(ke == 1))
            nc.scalar.activation(o2T_sb[:], psum[:], AF.Relu,
                                 bias=b2T_sb[:, 0:1])
            psum3 = ps4.tile([P, BC], F32, tag="mm")
            nc.tensor.matmul(psum3[0:64, :], w3_sb[:], o2T_sb[:], start=True, stop=True)
            nc.scalar.activation(o3T_sb[:], psum3[0:64, :], AF.Relu, bias=b3T_sb[:, 0:1])
            psum4 = ps4.tile([P, BC], F32, tag="mm")
            nc.tensor.matmul(psum4[0:1, :], w4_sb[:], o3T_sb[:], start=True, stop=True)
            # sigmoid(x) = 1/(1+exp(-x)) via the resident exp table: avoids
            # a 1.3us ACT_TABLE_LOAD for AF.Sigmoid on the critical tail.
            en_sb = acts.tile([1, BC], F32)
            nc.scalar.activation(en_sb[:], psum4[0:1, :], AF.Exp,
                                 bias=b4n_sb[:, 0:1], scale=-1.0)
            nc.vector.tensor_scalar_add(en_sb[:], en_sb[:], 1.0)
            nc.vector.reciprocal(outT_sb[:], en_sb[:])
            # one contiguous 128B row (a [32,1] store is 32 tiny
            # packets and gates kernel teardown by ~2us)
            nc.sync.dma_start(out[0:1, :], outT_sb[0:1, :])

    nc.compile()
    return nc


_BF = ml_dtypes.bfloat16
_F8 = ml_dtypes.float8_e4m3


def _q8(a, scale):
    return np.clip(np.asarray(a, np.float32) * scale, -240.0, 240.0).astype(_F8)


def _mbl():
    m = np.zeros((GS, P), np.float32)
    for j in range(GS):
        m[j, j * L:(j + 1) * L] = 1.0
    return m.astype(_BF)


def _mbr():
    m = np.full((GS, P), -1600.0 * SQK * SQK, np.float32)
    for j in range(GS):
        m[j, j * L:(j + 1) * L] = 0.0
    return m.astype(_BF)


def _prep_shared(w_red, b_red, w_in, b_in, w_out, b_out, w1, b1, w2, b2, w3, b3,
                 w4, b4):
    f32 = np.float32
    w_red, w_in, w_out = (np.asarray(a, f32) for a in (w_red, w_in, w_out))
    w1 = np.asarray(w1, f32)
    b_in = np.asarray(b_in, f32)
    b_out_eff = np.asarray(b_out, f32) + w_out @ b_in[2 * E:3 * E]
    w1_eff = w1 @ w_out                     # (256, E)
    b1_eff = np.asarray(b1, f32) + w1 @ b_out_eff
    w8 = _q8(w_red.T, WRS)          # [H, E]
    wredS = np.ascontiguousarray(
        w8.reshape(KX, P, KE // 2, 2 * P).transpose(1, 2, 0, 3)
        .reshape(P, -1))
    win8 = _q8(w_in.T, WIS)         # [E, 3E]
    def _stripe4(block):            # [E, 2048] -> [P, 4*KE*512]
        return np.ascontiguousarray(
            block.reshape(KE, P, 4, 512).transpose(1, 2, 0, 3).reshape(P, -1))
    w1Sa = np.ascontiguousarray(
        w1_eff.T.astype(_BF).reshape(KE, P, 256).transpose(1, 0, 2)
        .reshape(P, -1))
    shared = {
        "wredS": wredS,
        "winSq": _stripe4(win8[:, :E]),
        "winSk": _stripe4(win8[:, E:2 * E]),
        "winSv": _stripe4(win8[:, 2 * E:]),
        "w1S": w1Sa,
        "w2T": np.ascontiguousarray(np.asarray(w2, f32).T).astype(_BF),
        "w3T": np.ascontiguousarray(np.asarray(w3, f32).T).astype(_BF),
        "w4T": np.ascontiguousarray(np.asarray(w4, f32).T).astype(_BF),
        "bredT16": np.ascontiguousarray(
            (np.asarray(b_red, f32) * HS).reshape(KE, P).T),
        "binT": np.ascontiguousarray((b_in[:2 * E] * SQK).reshape(2 * KE, P).T),
        "b1effT": np.ascontiguousarray(b1_eff.reshape(2, P).T),
        "b2T": np.ascontiguousarray(np.asarray(b2, f32).reshape(1, P).T),
        "b3T": np.ascontiguousarray(np.asarray(b3, f32).reshape(1, 64).T),
        "b4": np.asarray(b4, f32).reshape(1, 1),
        "mbl": _mbl(), "mbr": _mbr(),
    }
    return shared


def kernel(x, w_red, b_red, w_in, b_in, w_out, b_out, w1, b1, w2, b2, w3, b3,
           w4, b4):
    global LAST_EXEC_TIME_NS
    x = np.asarray(x, np.float32)
    shared = _prep_shared(w_red, b_red, w_in, b_in, w_out, b_out, w1, b1, w2,
                          b2, w3, b3, w4, b4)
    in_maps = []
    for c in range(NCORES):
        xc = x[c * BC:(c + 1) * BC].reshape(M, H)
        x8 = _q8(xc.T, XS)  # [H, M]
        xSc = np.ascontiguousarray(
            x8.reshape(KX, P, M).transpose(1, 0, 2).reshape(P, -1))
        in_maps.append({"xS": xSc, **shared})
    nc = _build_kernel()
    trace = os.environ.get("BASS_TRACE", "0") == "1"
    kw = {}
    if trace:
        _install_ntff_hook_shim()
        import concourse.bass_utils as _bu
        _bu.upload_artifacts = lambda d: str(d)  # no artifact bucket here
        tmpdir = os.environ.get("BASS_TRACE_DIR", "/tmp/bass_trace")
        os.makedirs(tmpdir, exist_ok=True)
        kw = {"trace": True, "tmpdir": tmpdir}
    res = run_bass_kernel_spmd(nc, in_maps, core_ids=list(range(NCORES)), **kw)
    LAST_EXEC_TIME_NS = res.exec_time_ns
    return np.concatenate([res.results[c]["out"].reshape(BC, 1)
                           for c in range(NCORES)], axis=0)


if __name__ == "__main__":
    print("smoke test: building kernel only")
    _build_kernel()
    print("build OK")



# revision 53
# speedup vs baseline: 1.0216x; 1.0074x over previous
"""Trainium2 Bass kernel for AttentionMLPReduction (fp8 DoubleRow version).

Reference computation (per sample, B=256, L=32, H=4096, E=2048, NH=8, hd=256):
  h    = relu(x @ w_red.T + b_red)                  (B,L,E)
  qkv  = h @ w_in.T + b_in ; q,k,v = split(qkv)
  attn = softmax(q @ k.T / sqrt(hd))  per head      (B,NH,L,L)
  ctx  = attn @ v                                   (B,NH,L,hd) -> (B,L,E)
  attn_output = ctx @ w_out.T + b_out               (B,L,E)
  w_mean = attn.mean(heads); w_norm = w_mean / rowsum  (== w_mean)
  pooled = mean_q(w_norm @ attn_output)             (B,E)
  out = sigmoid(mlp(pooled))                        (B,1)

Algebraic simplifications:
  * w_norm == w_mean exactly (rows already sum to 1).
  * pooled[b] = u[b] @ attn_output[b] with u[b,l] = mean_q w_mean[b,q,l].
  * z[b] := u[b] @ ctx[b]; pooled = z @ w_out.T + b_out_eff with
    b_out_eff = b_out + w_out @ b_in_v (since sum_l u[b,l] == 1).
  * w_out is folded into the MLP head entirely:
      o1 = relu(pooled @ w1.T + b1) = relu(z @ (w1 @ w_out).T + b1_eff),
      b1_eff = b1 + w1 @ b_out_eff, so the (B,E)x(E,E) GEMM3 disappears.

Precision: the three big GEMMs (x@w_red, h@w_qk, h@w_v) run in fp8 e4m3 with
MatmulPerfMode.DoubleRow (2 k-rows per PE pass). Host pre-scales operands to
the e4m3 sweet range with power-of-2 factors (exact), and the dequant scales
fold into the psum->sbuf activation step:
  x*32, w_red*4096 -> h8 = 16*h via relu scale 1/8192
  h8 (=16*h), w_in*8192 -> q,k,v via scale 1/131072
Softmax block-diag masking is folded into the scores psum as a rank-4 matmul
driving off-diagonal 32x32 blocks to exp(-100) == 0. Schedule: GEMM1 (DMA
issue order tuned so x/wred sub-tiles arrive just-in-time), v GEMM, then per
head-pair qk GEMMs with the previous pair's attention interleaved 1:1 into
the fp8 streams; z is fused into the last pair's tails; the final sigmoid
avoids an ACT_TABLE_LOAD by using exp + reciprocal.

Sharding: pure data parallel over batch; 32 samples per core, weights
replicated (prepared/cast once on the host).

Per-core layouts (partition dim first):
  xT8   [H=4096, M=1024] fp8   (M = 32 samples x L=32), value = 32*x
  hT    [E=2048, M]      fp8   in SBUF, value = 16*h
  qT,kT per head-pair: [128, 4*M] fp8 (col = dtile*M + m), value = 32*q
    -> scores run as one fp8-DR matmul over both d-tiles per group
  v     [M, E]           bf16   natural orientation
  ctx   [M, E]           bf16
  zT    [E, 32]          bf16 -> o1T [256,32] -> ... -> outT [1,32]
"""

import os
import numpy as np
import ml_dtypes

import concourse.bass as bass
import concourse.mybir as mybir
import concourse.tile as tile
from concourse import bacc
from concourse.bass_utils import run_bass_kernel_spmd
from concourse.masks import make_identity

BF16 = mybir.dt.bfloat16
F8 = mybir.dt.float8e4
F32 = mybir.dt.float32
AF = mybir.ActivationFunctionType
DR = mybir.MatmulPerfMode.DoubleRow

B, L, H, E, NH = 256, 32, 4096, 2048, 8
HD = E // NH  # 256
NCORES = 8
BC = B // NCORES  # 32 samples per core
M = BC * L  # 1024 rows per core
P = 128
KX = H // P  # 32 k-tiles for GEMM1
KE = E // P  # 16 k-tiles for E-contraction GEMMs
MT = M // P  # 8 m-tiles
GS = P // L  # 4 samples per partition-tile

# fp8 scaling (all powers of two -> exact to fold/unfold)
XS = 32.0        # x pre-scale
WRS = 4096.0     # w_red pre-scale
HS = 16.0        # h post-scale (stored h8 = HS*h)
WIS = 8192.0     # w_in pre-scale
G1_SCALE = HS / (XS * WRS)      # psum -> h8
QKV_SCALE = 1.0 / (HS * WIS)    # psum -> q/k/v
SQK = 32.0   # q/k post-scale: qT/kT stored fp8 as 32*q (|q|<3.4 -> <109)

# module-level stash for the last run's HW exec time (ns), if traced
LAST_EXEC_TIME_NS = None


def _install_ntff_hook_shim():
    """antenv.axon_hooks is missing in this container; bass_utils imports it
    when trace=True under axon. Recreate it and register the ctypes-driven
    NRT profile hook from trn_boot if available."""
    import sys
    import types
    try:
        from antenv import axon_hooks  # noqa: F401
        return
    except ImportError:
        pass
    try:
        import antenv
    except ImportError:
        return
    m = types.ModuleType("antenv.axon_hooks")
    m._hook = None
    m.set_axon_ntff_profile_hook = lambda h: setattr(m, "_hook", h)
    m.get_axon_ntff_profile_hook = lambda: m._hook
    sys.modules["antenv.axon_hooks"] = m
    antenv.axon_hooks = m
    try:
        from trn_agent_boot.trn_boot import _ntff_profile_via_ctypes
        hook = _ntff_profile_via_ctypes("/opt/axon/libaxon_pjrt.so")
        if hook is not None:
            m._hook = hook
    except Exception:
        pass


def _build_kernel() -> bass.Bass:
    nc = bacc.Bacc(None, target_bir_lowering=False, debug=False)

    # ---- DRAM parameters (per-core shard views) ----
    # pre-striped on host: every row below is the exact SBUF line for one
    # partition, so DMAs are contiguous 8-16KB lines at full HBM speed.
    xS = nc.dram_tensor("xS", [P, KX * M], F8, kind="ExternalInput")
    wredS = nc.dram_tensor("wredS", [P, (KE // 2) * KX * 2 * P], F8,
                           kind="ExternalInput")
    winSq = nc.dram_tensor("winSq", [P, 4 * KE * 512], F8, kind="ExternalInput")
    winSk = nc.dram_tensor("winSk", [P, 4 * KE * 512], F8, kind="ExternalInput")
    winSv = nc.dram_tensor("winSv", [P, 4 * KE * 512], F8, kind="ExternalInput")
    w1S = nc.dram_tensor("w1S", [P, KE * 256], BF16, kind="ExternalInput")
    w2T = nc.dram_tensor("w2T", [256, P], BF16, kind="ExternalInput")
    w3T = nc.dram_tensor("w3T", [P, 64], BF16, kind="ExternalInput")
    w4T = nc.dram_tensor("w4T", [64, 1], BF16, kind="ExternalInput")
    bredT16 = nc.dram_tensor("bredT16", [P, KE], F32, kind="ExternalInput")
    binT = nc.dram_tensor("binT", [P, 2 * KE], F32, kind="ExternalInput")
    b1effT = nc.dram_tensor("b1effT", [P, 2], F32, kind="ExternalInput")
    b2T = nc.dram_tensor("b2T", [P, 1], F32, kind="ExternalInput")
    b3T = nc.dram_tensor("b3T", [64, 1], F32, kind="ExternalInput")
    b4 = nc.dram_tensor("b4", [1, 1], F32, kind="ExternalInput")
    mbl = nc.dram_tensor("mbl", [GS, P], BF16, kind="ExternalInput")
    mbr = nc.dram_tensor("mbr", [GS, P], BF16, kind="ExternalInput")
    out = nc.dram_tensor("out", [1, BC], F32, kind="ExternalOutput")

    from contextlib import ExitStack

    with tile.TileContext(nc) as tc, ExitStack() as ctx:
        const = ctx.enter_context(tc.tile_pool(name="const", bufs=1))
        bredT_sb = const.tile([P, KE], F32)
        binT_sb = const.tile([P, 2 * KE], F32)
        b1T_sb = const.tile([P, 2], F32)
        b2T_sb = const.tile([P, 1], F32)
        b3T_sb = const.tile([64, 1], F32)
        b4_sb = const.tile([1, 1], F32)
        b4n_sb = const.tile([1, 1], F32)
        ind_sb = const.tile([P, GS], BF16)
        ident_sb = const.tile([P, P], BF16)
        mbl_sb = const.tile([GS, P], BF16)
        mbr_sb = const.tile([GS, P], BF16)

        def _const_dmas():
            # consts ride the scalar ring BEHIND the x chunks: the gpsimd
            # software ring trickles at ~16KB/25us and was still delivering
            # b3/b4/mbl/mbr at ~350us -- gating the MLP tail stages and the
            # final drain.
            nc.scalar.dma_start(binT_sb[:], binT[:])
            nc.scalar.dma_start(b1T_sb[:], b1effT[:])
            nc.scalar.dma_start(b2T_sb[:], b2T[:])
            nc.scalar.dma_start(b3T_sb[:], b3T[:])
            nc.scalar.dma_start(b4_sb[:], b4[:])
            nc.vector.tensor_scalar_mul(b4n_sb[:], b4_sb[:], -1.0)
            # indicator[p, j] = 1.0 if p // 32 == j else 0 (for z block-sums)
            nc.any.memset(ind_sb[:], 0.0)
            for j in range(GS):
                nc.any.memset(ind_sb[j * L:(j + 1) * L, j:j + 1], 1.0)
            make_identity(nc, ident_sb)
            # rank-4 factors of the block-diag mask bias: -1600*SQK^2 off-
            # diagonal, 0 on-diagonal (scores psum += mbl.T @ mbr).
            nc.scalar.dma_start(mbl_sb[:], mbl[:])
            nc.scalar.dma_start(mbr_sb[:], mbr[:])

        # persistent activations (live across phases)
        acts = ctx.enter_context(tc.tile_pool(name="acts", bufs=1))
        hT_sb = acts.tile([P, KE * M], F8)         # col = et*M + m, = 16*h
        zT_sb = acts.tile([P, KE * BC], BF16)      # col = ec*BC + s
        o1T_sb = acts.tile([P, 2 * BC], BF16)
        o2T_sb = acts.tile([P, BC], BF16)
        o3T_sb = acts.tile([64, BC], BF16)
        outT_sb = acts.tile([1, BC], F32)

        # x + weights streamed early. Scalar ring (Q10): xT, v stripes, w1
        # family, then ALL qk pair stripes (issued after GEMM1 so they never
        # queue ahead of wred stripes). Sync ring (Q1): GEMM1 eg stripes
        # only, bufs=3 with issue-ahead so eg arrivals always lead compute.
        # Gpsimd ring starts earliest (~5us): it carries the first-needed
        # x/wred sub-tiles so the first matmul can fire ~7us in.
        w1p = ctx.enter_context(tc.tile_pool(name="w1p", bufs=1))
        w1s = w1p.tile([P, KE * 256], BF16)
        w2_sb = w1p.tile([P, 2 * P], BF16)
        w3_sb = w1p.tile([P, 64], BF16)
        w4_sb = w1p.tile([64, 1], BF16)
        EG = 2  # e-tiles per wred stripe

        def _eg_dma(eg):
            stripe = wpool.tile([P, KX * EG * P], F8, name=f"wrs{eg}",
                                tag="wrs")
            sz = KX * EG * P
            nc.sync.dma_start(stripe[:], wredS[:, eg * sz:(eg + 1) * sz])
            return stripe

        wqk = ctx.enter_context(tc.tile_pool(name="winqk", bufs=2))
        vpool = ctx.enter_context(tc.tile_pool(name="vctx", bufs=1))
        v_sb = vpool.tile([P, MT * E], BF16)    # col = mt*E + f
        ctx_sb = vpool.tile([P, MT * E], BF16)  # col = mt*E + e
        wv_cm = tc.tile_pool(name="winv", bufs=1)
        wv = wv_cm.__enter__()
        v_stripes = [wv.tile([P, KE * 512], F8, name=f"vst{fc}", tag=f"vst{fc}")
                     for fc in range(4)]
        # wred + x pools are innermost so both can close (LIFO) right after
        # GEMM1, freeing 48KB/partition for the attention-phase pools.
        wpool_cm = tc.tile_pool(name="wred", bufs=2)
        wpool = wpool_cm.__enter__()
        xpool_cm = tc.tile_pool(name="xT", bufs=1)
        xpool = xpool_cm.__enter__()
        xq = [xpool.tile([P, (KX // 4) * M], F8, name=f"xq{i}", tag=f"xq{i}")
              for i in range(4)]

        # --- scalar ring: bredT first (needed ~25us), then x in 0.5MB
        # halves so GEMM1's q-quarters never wait on a full 1MB chunk, then
        # the MLP tail weights (arrive ~25us; needed ~350us).
        QSZ = KE * 512
        XSZ = (KX // 4) * M
        for i in range(4):
            # kt-pair quarters for the first two chunks (finer arrival
            # granularity while DMA is oversubscribed), halves after.
            nsub = 4 if i < 2 else 2
            w = XSZ // nsub
            for hf in range(nsub):
                c0 = i * XSZ + hf * w
                nc.scalar.dma_start(xq[i][:, hf * w:(hf + 1) * w],
                                    xS[:, c0:c0 + w])
        nc.scalar.dma_start(
            w2_sb[:].rearrange("p (ke n) -> p ke n", ke=2),
            w2T[:].rearrange("(ke p) n -> p ke n", p=P))
        nc.scalar.dma_start(w3_sb[:], w3T[:])
        nc.scalar.dma_start(w4_sb[:], w4T[:])
        _const_dmas()
        # --- sync ring: wred ONLY during GEMM1 (q0 quarter first so the
        # first matmuls wait on 256KB, not 1MB); v/w1 queue BEHIND the eg
        # stripes (emitted after the GEMM1 loop) so they can never starve
        # the wred stream mid-GEMM.
        wrs0 = wpool.tile([P, KX * EG * P], F8, name="wrs0", tag="wrs")
        kw = 8 * EG * P  # wred cols for kt0-7 (the q=0 quarter)
        nc.sync.dma_start(wrs0[:, 0:kw], wredS[:, 0:kw])
        nc.sync.dma_start(wrs0[:, kw:KX * EG * P], wredS[:, kw:KX * EG * P])
        # bredT (64B rows, packet-slow) hides behind wrs0 on the sync ring,
        # done ~14us, needed ~30us.
        nc.sync.dma_start(bredT_sb[:], bredT16[:])
        eg_stripes = {0: wrs0, 1: _eg_dma(1)}
        qk_stripes = {}  # (hp, 'qs'/'ks') -> tile; filled after GEMM1

        with ExitStack() as s1:
            xq3 = [t[:].rearrange("p (kt m) -> p kt m", kt=KX // 4) for t in xq]
            ps1 = s1.enter_context(tc.tile_pool(name="ps1", bufs=2, space="PSUM"))
            for eg in range(KE // EG):
                stripe = eg_stripes[eg]
                if eg + 2 < KE // EG:
                    eg_stripes[eg + 2] = _eg_dma(eg + 2)
                w3r = stripe[:].rearrange("p (kt e) -> p kt e", kt=KX)
                # 4 open psum groups (el, mc), accumulated in 4 quarter-k
                # passes so compute can start after the first x chunk lands.
                psums = {}
                for el in range(EG):
                    for mc in range(2):
                        psums[el, mc] = ps1.tile(
                            [P, 512], F32, name=f"g1ps{el}{mc}",
                            tag=f"g1ps{el}{mc}")
                for q in range(4):
                    for el in range(EG):
                        for mc in range(2):
                            for kp in range(4):
                                nc.tensor.matmul(
                                    psums[el, mc][:],
                                    w3r[:, q * 8 + 2 * kp:q * 8 + 2 * kp + 2,
                                        el * P:(el + 1) * P],
                                    xq3[q][:, 2 * kp:2 * kp + 2,
                                           mc * 512:(mc + 1) * 512],
                                    start=(q == 0 and kp == 0),
                                    stop=(q == 3 and kp == 3),
                                    perf_mode=DR)
                for el in range(EG):
                    et = eg * EG + el
                    for mc in range(2):
                        nc.scalar.activation(
                            hT_sb[:, et * M + mc * 512:et * M + (mc + 1) * 512],
                            psums[el, mc][:], AF.Relu,
                            bias=bredT_sb[:, et:et + 1], scale=G1_SCALE)
        xpool_cm.__exit__(None, None, None)  # xq dead after GEMM1
        wpool_cm.__exit__(None, None, None)  # wred stripes dead after GEMM1
        # v stripes + w1 ride the sync ring BEHIND the eg stripes (arrive
        # ~110us, needed at the v GEMM ~125us).
        for fc in range(4):
            nc.sync.dma_start(v_stripes[fc][:],
                              winSv[:, fc * QSZ:(fc + 1) * QSZ])
        nc.sync.dma_start(w1s[:], w1S[:])
        # qk pair-0/1 stripes on the now-idle scalar ring (arrive ~40us)
        for hp in range(2):
            for tag, src in (("qs", winSq), ("ks", winSk)):
                st = wqk.tile([P, KE * 512], F8, name=f"{tag}{hp}", tag=tag)
                nc.scalar.dma_start(st[:], src[:, hp * QSZ:(hp + 1) * QSZ])
                qk_stripes[(hp, tag)] = st
        h3 = hT_sb[:].rearrange("p (ke m) -> p ke m", ke=KE)

        # -------- qk GEMMs (all 4 pairs), then v GEMM + ALL attention ------
        # Attention needs v only as ctx-matmul rhs, so the entire attention
        # pipeline (scores/exp/softmax/u/ctx) interleaves into the v GEMM's
        # fp8 stream; z matmuls are deferred to one dense batch at the end
        # (back-to-back z mms pipeline at ~26ns each).
        with ExitStack() as s2:
            # ---- v GEMM first (attention ctx needs v); its psum pool
            # closes before the attention pools open.
            def _v_unit(psv, mt, fc):
                ps = psv.tile([P, 512], F32, name="vps", tag="vps")
                st3 = v_stripes[fc][:].rearrange("p (ke f) -> p ke f", ke=KE)
                for kp in range(KE // 2):
                    nc.tensor.matmul(
                        ps[:],
                        h3[:, 2 * kp:2 * kp + 2, mt * P:(mt + 1) * P],
                        st3[:, 2 * kp:2 * kp + 2, :],
                        start=(kp == 0), stop=(kp == KE // 2 - 1),
                        perf_mode=DR)
                dst = v_sb[:, mt * E + fc * 512:mt * E + (fc + 1) * 512]
                if fc % 2 == 0:
                    nc.scalar.activation(dst, ps[:], AF.Copy,
                                         scale=QKV_SCALE)
                else:
                    nc.vector.tensor_scalar_mul(dst, ps[:], QKV_SCALE)

            with ExitStack() as s2v:
                psv = s2v.enter_context(
                    tc.tile_pool(name="psv", bufs=4, space="PSUM"))
                for mt in range(MT):
                    for fc in range(4):
                        _v_unit(psv, mt, fc)
            wv_cm.__exit__(None, None, None)  # v stripes dead after v GEMM

            with ExitStack() as s2b:
                qk_out = s2b.enter_context(tc.tile_pool(name="qkT", bufs=4))
                psqk = s2b.enter_context(tc.tile_pool(name="psqk", bufs=2, space="PSUM"))
                psc = s2b.enter_context(tc.tile_pool(name="psc", bufs=2, space="PSUM"))
                psu = s2b.enter_context(tc.tile_pool(name="psu", bufs=1, space="PSUM"))
                attp = s2b.enter_context(tc.tile_pool(name="attp", bufs=4))
                atq = s2b.enter_context(tc.tile_pool(name="atq", bufs=3))
                # u accumulated across all heads in one psum bank; column
                # g's accumulation group spans h=0..7.
                psu_all = psu.tile([P, MT], F32, name="psu_all")

                def _qk_units(hp):
                    # 8 emission units (2 dst x 4 dl) for head-pair hp
                    if hp < 2:
                        q_stripe = qk_stripes[(hp, "qs")]
                        k_stripe = qk_stripes[(hp, "ks")]
                    else:
                        q_stripe = wqk.tile([P, KE * 512], F8, tag="qs")
                        k_stripe = wqk.tile([P, KE * 512], F8, tag="ks")
                        nc.scalar.dma_start(q_stripe[:],
                                            winSq[:, hp * QSZ:(hp + 1) * QSZ])
                        nc.scalar.dma_start(k_stripe[:],
                                            winSk[:, hp * QSZ:(hp + 1) * QSZ])
                    # qT2/kT2: col = dl*M + m, dl 0..3 (dtile = 4*hp + dl)
                    # stored fp8 (= SQK*q): scores run as ONE DoubleRow
                    # matmul per group, and all 4 pairs fit in SBUF.
                    qT_sb = qk_out.tile([P, 4 * M], F8, tag="qT")
                    kT_sb = qk_out.tile([P, 4 * M], F8, tag="kT")
                    units = []
                    for dst, stripe, bcol0 in ((qT_sb, q_stripe, 4 * hp),
                                               (kT_sb, k_stripe, KE + 4 * hp)):
                        s3 = stripe[:].rearrange("p (ke f) -> p ke f", ke=KE)
                        for dl in range(4):
                            def _u(dst=dst, s3=s3, bcol0=bcol0, dl=dl):
                                psums = [psqk.tile([P, 512], F32,
                                                   name=f"qkps{i}",
                                                   tag=f"qkps{i}")
                                         for i in range(2)]
                                for kp in range(KE // 2):
                                    for mc in range(2):
                                        nc.tensor.matmul(
                                            psums[mc][:],
                                            s3[:, 2 * kp:2 * kp + 2, dl * P:(dl + 1) * P],
                                            h3[:, 2 * kp:2 * kp + 2, mc * 512:(mc + 1) * 512],
                                            start=(kp == 0),
                                            stop=(kp == KE // 2 - 1),
                                            perf_mode=DR)
                                for mc in range(2):
                                    d_ap = dst[:, dl * M + mc * 512:dl * M + (mc + 1) * 512]
                                    if mc == 0:
                                        nc.scalar.activation(
                                            d_ap, psums[mc][:], AF.Identity,
                                            bias=binT_sb[:, bcol0 + dl:bcol0 + dl + 1],
                                            scale=QKV_SCALE * SQK)
                                    else:
                                        nc.vector.tensor_scalar(
                                            out=d_ap, in0=psums[mc][:],
                                            scalar1=QKV_SCALE * SQK,
                                            scalar2=binT_sb[:, bcol0 + dl:bcol0 + dl + 1],
                                            op0=mybir.AluOpType.mult,
                                            op1=mybir.AluOpType.add)
                            units.append(_u)
                    return units, qT_sb, kT_sb

                qkT = []

                # ---- v GEMM units: (mt, fc) granularity, fc-outer so only
                # one psum group is open at a time (2 banks via rotation).
                # ---- attention S/T closures per (head, group-batch) ----
                zref = {}

                def _mk_head(hp, hh):
                    h = 2 * hp + hh
                    d0 = 2 * hh
                    qT_sb, kT_sb = qkT[hp]
                    qv3 = qT_sb[:].rearrange("p (dl m) -> p dl m", dl=4)
                    kv3 = kT_sb[:].rearrange("p (dl m) -> p dl m", dl=4)

                    def _scores(gb):
                        g0 = gb * 4
                        ps_s4 = psqk.tile([P, 4 * P], F32,
                                          name=f"s4_{h}_{gb}", tag="qkps0")
                        for j in range(4):
                            g = g0 + j
                            sl = ps_s4[:, j * P:(j + 1) * P]
                            nc.tensor.matmul(
                                sl,
                                qv3[:, d0:d0 + 2, g * P:(g + 1) * P],
                                kv3[:, d0:d0 + 2, g * P:(g + 1) * P],
                                start=True, stop=False, perf_mode=DR)
                            nc.tensor.matmul(sl, mbl_sb[:], mbr_sb[:],
                                             start=False, stop=True)
                        # bd4 = block-diag masked exp(scores/sqrt(hd))
                        bd4 = attp.tile([P, 4 * P], BF16, tag="bd4")
                        nc.scalar.activation(
                            bd4[:], ps_s4[:], AF.Exp,
                            scale=1.0 / (np.sqrt(HD) * SQK * SQK))
                        S4 = attp.tile([P, 4], F32, tag="S4")
                        nc.vector.tensor_reduce(
                            S4[:],
                            bd4[:].rearrange("p (j q) -> p j q", j=4),
                            axis=mybir.AxisListType.X,
                            op=mybir.AluOpType.add)
                        rS4 = attp.tile([P, 4], F32, tag="rS4")
                        nc.vector.reciprocal(rS4[:], S4[:])
                        rS4b = attp.tile([P, 4], BF16, tag="rS4b")
                        nc.vector.tensor_copy(rS4b[:], rS4[:])
                        return bd4, rS4, rS4b

                    def _tail(st, gb):
                        bd4, rS4, rS4b = st
                        g0 = gb * 4
                        bdT4_ps = psqk.tile([P, 4 * P], BF16,
                                            name=f"bdT4_{h}_{gb}",
                                            tag="qkps1")
                        for j in range(4):
                            nc.tensor.transpose(
                                bdT4_ps[:, j * P:(j + 1) * P],
                                bd4[:, j * P:(j + 1) * P], ident_sb[:])
                        bdT4 = atq.tile([P, 4 * P], BF16, tag="bdT_sb")
                        nc.vector.tensor_copy(bdT4[:], bdT4_ps[:])
                        # u matmuls fill the bdT copy latency
                        for j in range(4):
                            nc.tensor.matmul(
                                psu_all[:, g0 + j:g0 + j + 1],
                                bd4[:, j * P:(j + 1) * P],
                                rS4b[:, j:j + 1],
                                start=(h == 0), stop=(h == NH - 1))
                        for jp in range(2):
                            ps_ctx2 = psc.tile([P, 2 * HD], F32, tag="ctx")
                            for jj in range(2):
                                j = jp * 2 + jj
                                g = g0 + j
                                nc.tensor.matmul(
                                    ps_ctx2[:, jj * HD:(jj + 1) * HD],
                                    bdT4[:, j * P:(j + 1) * P],
                                    v_sb[:, g * E + h * HD:g * E + (h + 1) * HD],
                                    start=True, stop=True)
                            for jj in range(2):
                                j = jp * 2 + jj
                                g = g0 + j
                                # normalized ctx copy+scale. During the sp
                                # phases scalar is loaded with qk acts, so
                                # it rides vector; in the exposed hp3 flush
                                # vector IS the bottleneck (its in-order
                                # queue stalls the next head's softmax), so
                                # it moves to the near-idle scalar engine.
                                dst = ctx_sb[:, g * E + h * HD:g * E + (h + 1) * HD]
                                if hp == NH // 2 - 1:
                                    nc.scalar.activation(
                                        dst, ps_ctx2[:, jj * HD:(jj + 1) * HD],
                                        AF.Copy, scale=rS4[:, j:j + 1])
                                else:
                                    nc.vector.tensor_scalar_mul(
                                        dst, ps_ctx2[:, jj * HD:(jj + 1) * HD],
                                        rS4[:, j:j + 1])
                        if h == NH - 1:
                            # psu column complete: z fused here so the
                            # back-to-back z mms overlap the attn pipeline
                            for j in range(4):
                                g = g0 + j
                                ind_u = attp.tile([P, GS], BF16, tag="iu")
                                nc.vector.tensor_scalar(
                                    out=ind_u[:], in0=ind_sb[:],
                                    scalar1=psu_all[:, g:g + 1],
                                    scalar2=1.0 / (L * NH),
                                    op0=mybir.AluOpType.mult,
                                    op1=mybir.AluOpType.mult)
                                for ec in range(KE):
                                    nc.tensor.matmul(
                                        zref['t'][:, ec * BC + g * GS:ec * BC + (g + 1) * GS],
                                        ctx_sb[:, g * E + ec * P:g * E + (ec + 1) * P],
                                        ind_u[:], start=True, stop=True)

                    stash = {}

                    def S(gb):
                        stash[gb] = _scores(gb)

                    def T(gb):
                        _tail(stash.pop(gb), gb)

                    return S, T

                pend = []
                for hp in range(NH // 2):
                    qku, qT_sb, kT_sb = _qk_units(hp)
                    qkT.append((qT_sb, kT_sb))
                    for i, u in enumerate(qku):
                        u()
                        if i < len(pend):
                            pend[i]()
                    for u in pend[len(qku):]:
                        u()
                    S0, T0 = _mk_head(hp, 0)
                    S1, T1 = _mk_head(hp, 1)
                    pend = [lambda: S0(0), lambda: S0(1),
                            lambda: T0(0), lambda: T0(1),
                            lambda: S1(0), lambda: S1(1),
                            lambda: T1(0), lambda: T1(1)]
                    if hp == NH // 2 - 1:
                        # last pair flushes with nothing to hide behind:
                        # round-robin S/T across both heads for deeper
                        # cross-unit pipelining. z accumulates into its own
                        # psu-pool bank.
                        zref['t'] = psu.tile([P, KE * BC], F32,
                                             name="ps_zT")
                for u in pend:
                    u()

                nc.vector.tensor_copy(zT_sb[:], zref['t'][:])

        # ---------------- MLP head (w_out folded into w1eff) ----------------
        with ExitStack() as s4:
            ps4 = s4.enter_context(tc.tile_pool(name="ps4", bufs=4, space="PSUM"))
            w1s3 = w1s[:].rearrange("p (ke n) -> p ke n", ke=KE)
            for nt in range(2):
                psum = ps4.tile([P, BC], F32, tag="mm")
                for ke in range(KE):
                    nc.tensor.matmul(psum[:], w1s3[:, ke, nt * P:(nt + 1) * P],
                                     zT_sb[:, ke * BC:(ke + 1) * BC],
                                     start=(ke == 0), stop=(ke == KE - 1))
                nc.scalar.activation(o1T_sb[:, nt * BC:(nt + 1) * BC], psum[:],
                                     AF.Relu, bias=b1T_sb[:, nt:nt + 1])
            psum = ps4.tile([P, BC], F32, tag="mm")
            w2s3 = w2_sb[:].rearrange("p (ke n) -> p ke n", ke=2)
            for ke in range(2):
                nc.tensor.matmul(psum[:], w2s3[:, ke, :],
                                 o1T_sb[:, ke * BC:(ke + 1) * BC],
                                 start=(ke == 0), stop=(ke == 1))
            nc.scalar.activation(o2T_sb[:], psum[:], AF.Relu,
                                 bias=b2T_sb[:, 0:1])
            psum3 = ps4.tile([P, BC], F32, tag="mm")
            nc.tensor.matmul(psum3[0:64, :], w3_sb[:], o2T_sb[:], start=True, stop=True)
            nc.scalar.activation(o3T_sb[:], psum3[0:64, :], AF.Relu, bias=b3T_sb[:, 0:1])
            psum4 = ps4.tile([P, BC], F32, tag="mm")
            nc.tensor.matmul(psum4[0:1, :], w4_sb[:], o3T_sb[:], start=True, stop=True)
            # sigmoid(x) = 1/(1+exp(-x)) via the resident exp table: avoids
            # a 1.3us ACT_TABLE_LOAD for AF.Sigmoid on the critical tail.
            en_sb = acts.tile([1, BC], F32)
            nc.scalar.activation(en_sb[:], psum4[0:1, :], AF.Exp,
                                 bias=b4n_sb[:, 0:1], scale=-1.0)
            nc.vector.tensor_scalar_add(en_sb[:], en_sb[:], 1.0)
            nc.vector.reciprocal(outT_sb[:], en_sb[:])
            # one contiguous 128B row (a [32,1] store is 32 tiny
            # packets and gates kernel teardown by ~2us)
            nc.sync.dma_start(out[0:1, :], outT_sb[0:1, :])

    nc.compile()
    return nc


_BF = ml_dtypes.bfloat16
_F8 = ml_dtypes.float8_e4m3


def _q8(a, scale):
    return np.clip(np.asarray(a, np.float32) * scale, -240.0, 240.0).astype(_F8)


def _mbl():
    m = np.zeros((GS, P), np.float32)
    for j in range(GS):
        m[j, j * L:(j + 1) * L] = 1.0
    return m.astype(_BF)


def _mbr():
    m = np.full((GS, P), -1600.0 * SQK * SQK, np.float32)
    for j in range(GS):
        m[j, j * L:(j + 1) * L] = 0.0
    return m.astype(_BF)


def _prep_shared(w_red, b_red, w_in, b_in, w_out, b_out, w1, b1, w2, b2, w3, b3,
                 w4, b4):
    f32 = np.float32
    w_red, w_in, w_out = (np.asarray(a, f32) for a in (w_red, w_in, w_out))
    w1 = np.asarray(w1, f32)
    b_in = np.asarray(b_in, f32)
    b_out_eff = np.asarray(b_out, f32) + w_out @ b_in[2 * E:3 * E]
    w1_eff = w1 @ w_out                     # (256, E)
    b1_eff = np.asarray(b1, f32) + w1 @ b_out_eff
    w8 = _q8(w_red.T, WRS)          # [H, E]
    wredS = np.ascontiguousarray(
        w8.reshape(KX, P, KE // 2, 2 * P).transpose(1, 2, 0, 3)
        .reshape(P, -1))
    win8 = _q8(w_in.T, WIS)         # [E, 3E]
    def _stripe4(block):            # [E, 2048] -> [P, 4*KE*512]
        return np.ascontiguousarray(
            block.reshape(KE, P, 4, 512).transpose(1, 2, 0, 3).reshape(P, -1))
    w1Sa = np.ascontiguousarray(
        w1_eff.T.astype(_BF).reshape(KE, P, 256).transpose(1, 0, 2)
        .reshape(P, -1))
    shared = {
        "wredS": wredS,
        "winSq": _stripe4(win8[:, :E]),
        "winSk": _stripe4(win8[:, E:2 * E]),
        "winSv": _stripe4(win8[:, 2 * E:]),
        "w1S": w1Sa,
        "w2T": np.ascontiguousarray(np.asarray(w2, f32).T).astype(_BF),
        "w3T": np.ascontiguousarray(np.asarray(w3, f32).T).astype(_BF),
        "w4T": np.ascontiguousarray(np.asarray(w4, f32).T).astype(_BF),
        "bredT16": np.ascontiguousarray(
            (np.asarray(b_red, f32) * HS).reshape(KE, P).T),
        "binT": np.ascontiguousarray((b_in[:2 * E] * SQK).reshape(2 * KE, P).T),
        "b1effT": np.ascontiguousarray(b1_eff.reshape(2, P).T),
        "b2T": np.ascontiguousarray(np.asarray(b2, f32).reshape(1, P).T),
        "b3T": np.ascontiguousarray(np.asarray(b3, f32).reshape(1, 64).T),
        "b4": np.asarray(b4, f32).reshape(1, 1),
        "mbl": _mbl(), "mbr": _mbr(),
    }
    return shared


def kernel(x, w_red, b_red, w_in, b_in, w_out, b_out, w1, b1, w2, b2, w3, b3,
           w4, b4):
    global LAST_EXEC_TIME_NS
    x = np.asarray(x, np.float32)
    shared = _prep_shared(w_red, b_red, w_in, b_in, w_out, b_out, w1, b1, w2,
                          b2, w3, b3, w4, b4)
    in_maps = []
    for c in range(NCORES):
        xc = x[c * BC:(c + 1) * BC].reshape(M, H)
        x8 = _q8(xc.T, XS)  # [H, M]
        xSc = np.ascontiguousarray(
            x8.reshape(KX, P, M).transpose(1, 0, 2).reshape(P, -1))
        in_maps.append({"xS": xSc, **shared})
    nc = _build_kernel()
    trace = os.environ.get("BASS_TRACE", "0") == "1"
    kw = {}
    if trace:
        _install_ntff_hook_shim()
        import concourse.bass_utils as _bu
        _bu.upload_artifacts = lambda d: str(d)  # no artifact bucket here
        tmpdir = os.environ.get("BASS_TRACE_DIR", "/tmp/bass_trace")
        os.makedirs(tmpdir, exist_ok=True)
        kw = {"trace": True, "tmpdir": tmpdir}
    res = run_bass_kernel_spmd(nc, in_maps, core_ids=list(range(NCORES)), **kw)
    LAST_EXEC_TIME_NS = res.exec_time_ns
    return np.concatenate([res.results[c]["out"].reshape(BC, 1)
                           for c in range(NCORES)], axis=0)


if __name__ == "__main__":
    print("smoke test: building kernel only")
    _build_kernel()
    print("build OK")

